# revision 21
# baseline (speedup 1.0000x reference)
"""Trainium2 Bass kernel for nn_DecoderBlock (B=2, S=2048, D=1024, DFF=4096, H=16).

Sharding: 8 cores = 2 batches x 4 cores. Each core owns 512 interleaved
128-token q-tiles (core j of a batch owns global tiles j, j+4, j+8, j+12) but
receives the FULL batch sequence (feature-major, bf16) with each 512-token
block rotated so the core's own 128-token tile sits at position 0 of every
block. LayerNorm1 and the K projection are computed redundantly for the full
sequence on every core, which removes the K AllGather entirely (the cost
model charges 15us fixed latency per collective; four sliced 1MB gathers
serialized at ~41us each on the collective engine in the baseline). The only
remaining collective is the tiny per-tile V-column-sum AllGather (32KB).

The reference module's triu/transpose softmax degenerates mathematically:
    c[q,k] = 1/denom(q)            for k < q
           = exp(s[q,q])/denom(q)  for k == q
           = 0                     for k > q
    denom(q) = q + sum_{k>=q} exp(s[q,k])
so attention output = (prefix_sum(V) + exp(diag)*V_own) / denom, and V never
crosses cores - only per-128-row-tile V column sums.

Numerics: all big matmuls run in bf16 (weights shipped bf16 from host), PSUM
accumulation stays f32, LayerNorm statistics stay f32. The small [*,128]-free
matmuls of the numerator phase run bf16 (f32r below 256 free columns costs 4
cycles/row on TRN2). Score exp row-sums use the activation engine's fused
accumulator instead of DVE reductions.

LayerNorm gamma/beta are folded into the downstream weights on the host
(wk' = g1*wk, bk' = bk + be1@wk, etc.; the residual picks up g1 via the
own-token extraction and be1 via bo' = bo + be1), so the on-chip LN applies
only (x - mean) * rstd (two DVE/Pool passes). The V projection runs with
resident wv in a 2-PSUM-bank footprint and is emission-interleaved with the
score loop (together with V column sums and the diagonal pass) so the tensor
engine fills the activation-bound exp window.
"""
import sys

sys.path.insert(0, "/opt/trn_rl_repo")

import ml_dtypes
import numpy as np

from contextlib import ExitStack

import concourse.bass as bass
import concourse.mybir as mybir
import concourse.tile as tile
from concourse.bass_utils import run_bass_kernel_spmd
from concourse.vector_clock import ScopedClock

# ---------------------------------------------------------------------------
# Patch for this walrus build: it rejects more than one sync-wait command per
# instruction. Split multi-wait instructions into preceding same-engine NOPs
# (program order on the engine preserves semantics), both for scheduled
# instructions and for the TileContext tail drain.
# ---------------------------------------------------------------------------
_MAX_WAITS = 1
_orig_lower = tile.TileContext._lower_ordered_insts


def _split_waits(ordered):
    for bb_name, insts in ordered.items():
        new_insts = []
        for inst in insts:
            si = inst.sync_info
            if si is not None and si.on_wait and len(si.on_wait) > _MAX_WAITS:
                waits = list(si.on_wait)
                for i, w in enumerate(waits[:-_MAX_WAITS]):
                    new_insts.append(
                        mybir.InstNoOp(
                            name=f"{inst.name}-ws{i}",
                            sync_info=mybir.SyncInfo(on_wait=[w], on_update=[]),
                            bass_nofuse=True,
                            engine=inst.engine,
                        )
                    )
                inst.sync_info = mybir.SyncInfo(
                    on_wait=waits[-_MAX_WAITS:],
                    on_update=list(si.on_update) if si.on_update else [],
                )
            new_insts.append(inst)
        ordered[bb_name] = new_insts
    return ordered


def _lower_ordered_insts(self, ordered):
    return _orig_lower(self, _split_waits(ordered))


def _drain_and_barrier(self, tick_clock, wait_clock):
    nc = self.nc
    drain_inst = nc.sync.drain()
    wait_clock.add_sem_waits(
        drain_inst.ins, ScopedClock({None: tick_clock.global_clock})
    )
    si = drain_inst.ins.sync_info
    waits = list(si.on_wait) if si is not None else []
    if len(waits) > _MAX_WAITS:
        drain_inst.ins.sync_info = mybir.SyncInfo(
            on_wait=waits[:_MAX_WAITS],
            on_update=list(si.on_update) if si.on_update else [],
        )
        for i in range(_MAX_WAITS, len(waits), _MAX_WAITS):
            nop = nc.sync.nop(nofuse=True)
            nop.ins.sync_info = mybir.SyncInfo(
                on_wait=waits[i : i + _MAX_WAITS], on_update=[]
            )
    nc.all_engine_barrier()
    assert self.sems is not None
    popped = nc._tile_sem_poison_stack.pop()
    assert popped is self._sem_poison
    nc.clear_and_free_semaphores(list(self.sems.allocated().values()))
    nc.all_engine_barrier()


tile.TileContext._lower_ordered_insts = _lower_ordered_insts
tile.TileContext._drain_and_barrier = _drain_and_barrier

# ---------------------------------------------------------------------------

B, S, D, DFF, H = 2, 2048, 1024, 4096, 16
HD = D // H          # 64
EPS = 1e-5
NCORES = 8
CH = 4               # cores per batch / 128-tiles per 512-block
T = S // CH          # 512 own tokens per core
P = 128
NT = T // P          # 4 own q-tiles per core
NB = S // T          # 4 global 512-token blocks
DC = D // P          # 8 feature chunks

f32 = mybir.dt.float32
f32r = mybir.dt.float32r
bf16 = mybir.dt.bfloat16
AF = mybir.ActivationFunctionType
ALU = mybir.AluOpType
AX = mybir.AxisListType
NEG = -1.0e9


def _build():
    nc = bass.Bass(num_devices=NCORES)

    def par(name, shape, dt):
        return nc.declare_dram_parameter(name, shape, dt, isOutput=False)

    # per-core data
    xT_d = par("xT", [D, S], bf16)                # full batch, block-rotated
    qcount_d = par("qcount", [P, NT], f32)        # col i = global row index of tile i
    M1_d = par("M1", [P, T], f32r)                # boundary-block additive mask
    w32_d = par("w32", [32, 8], bf16)             # prefix tile-sum weights
    # shared weights (natural [din, dout] layout = lhsT), bf16
    wq_d = par("wq", [D, D], bf16)
    wk_d = par("wk", [D, D], bf16)
    wv_d = par("wv", [D, D], bf16)
    wo_d = par("wo", [D, D], bf16)
    w1_d = par("w1", [D, DFF], bf16)
    w2_d = par("w2", [DFF, D], bf16)
    # per-partition gamma/bias columns ([P, n_chunks], f32)
    g1_d = par("g1c", [P, DC], f32)
    bq_d = par("bqc", [P, DC], f32)
    bk_d = par("bkc", [P, DC], f32)
    bo_d = par("boc", [P, DC], f32)
    b1_d = par("b1c", [P, DFF // P], f32)
    b2_d = par("b2c", [P, DC], f32)
    bv_d = par("bvrow", [1, D], bf16)             # V bias as a row (free-dim)
    # shared constant matrices
    L128_d = par("L128", [P, P], bf16)            # L[k,q] = 1 if k < q
    I128b_d = par("I128b", [P, P], bf16)          # identity (bf16, NUM phase)
    I128r_d = par("I128r", [P, P], f32r)          # identity (f32r, mask matmul)
    ident_d = par("ident", [P, P], f32)
    H16T_d = par("H16T", [P, P], bf16)            # [:,16c+h]: head-h rows in chunk c
    H16b_d = par("H16b", [16, D], f32r)           # [h,128c+p]: head(p of chunk c)==h
    onesrow_d = par("onesrow", [1, S], bf16)
    onescol_d = par("onescol", [P, 1], bf16)

    out_d = nc.declare_dram_parameter("outT", [D, T], f32, isOutput=True)

    kv2_in = nc.dram_tensor("kv2_in", [8, T], bf16)
    kv2_out = nc.dram_tensor("kv2_out", [8 * CH, T], bf16)

    with tile.TileContext(nc, pool_alloc_mode="queue") as tc, ExitStack() as es:
            cp = es.enter_context(tc.tile_pool(name="cpool", bufs=1))
            lnp = es.enter_context(tc.tile_pool(name="lnstat", bufs=2))
            sp4 = es.enter_context(tc.tile_pool(name="small4", bufs=4))
            scr = es.enter_context(tc.tile_pool(name="scr", bufs=2))
            wp = es.enter_context(tc.tile_pool(name="wstream", bufs=10))
            es_ho = ExitStack()
            hop = es_ho.enter_context(tc.tile_pool(name="hopool", bufs=8, side="right"))

            # ---- load constants ----
            def load(pool, name, src, shape, dt, tag=None):
                t_ = pool.tile(shape, dt, tag=tag or name, name=tag or name)
                nc.sync.dma_start(t_[:], src[:])
                return t_

            qcount = load(cp, "qcount", qcount_d, [P, NT], f32)
            M1 = load(cp, "M1", M1_d, [P, T], f32r)
            w32 = load(cp, "w32", w32_d, [32, 8], bf16)
            g1c = load(cp, "g1c", g1_d, [P, DC], f32)
            bqc = load(cp, "bqc", bq_d, [P, DC], f32)
            bkc = load(cp, "bkc", bk_d, [P, DC], f32)
            boc = load(cp, "boc", bo_d, [P, DC], f32)
            b1c = load(cp, "b1c", b1_d, [P, DFF // P], f32)
            b2c = load(cp, "b2c", b2_d, [P, DC], f32)
            bvrow = load(cp, "bvrow", bv_d, [1, D], bf16)
            ident = load(cp, "ident", ident_d, [P, P], f32)
            H16T = load(cp, "H16T", H16T_d, [P, P], bf16)
            H16b = load(cp, "H16b", H16b_d, [16, D], f32r)
            onesrow = load(cp, "onesrow", onesrow_d, [1, S], bf16)
            onescol = load(cp, "onescol", onescol_d, [P, 1], bf16)
            L128 = load(cp, "L128", L128_d, [P, P], bf16)
            I128b = load(cp, "I128b", I128b_d, [P, P], bf16)
            I128r = load(cp, "I128r", I128r_d, [P, P], f32r)
            epsc = cp.tile([1, 1], f32, tag="epsc", name="epsc")
            nc.vector.memset(epsc[:], EPS)

            # ---- LayerNorm over feature-major [P, T]-blocked tiles ----
            # Produces plain (x - mean) * rstd; gamma/beta are folded into the
            # downstream weights on the host.
            def layer_norm(ps_pool, xin, nblk, out_pool, out_tag):
                outs = [[None] * nblk for _ in range(DC)]
                for blk in range(nblk):
                    sq = []
                    for k in range(DC):
                        s_ = scr.tile([P, T], bf16, tag="lnsq", name="lnsq", bufs=8)
                        nc.scalar.activation(s_[:], xin[k][blk][:], AF.Square)
                        sq.append(s_)
                    ps_sum = ps_pool.tile([1, T], f32, tag="ln_sum", name="ln_sum")
                    for k in range(DC):
                        nc.tensor.matmul(ps_sum[:], onescol[:], xin[k][blk][:],
                                         start=(k == 0), stop=(k == DC - 1))
                    ps_sq = ps_pool.tile([1, T], f32, tag="ln_sq", name="ln_sq")
                    for k in range(DC):
                        nc.tensor.matmul(ps_sq[:], onescol[:], sq[k][:],
                                         start=(k == 0), stop=(k == DC - 1))
                    mean = lnp.tile([1, T], f32, tag="ln_mean", name="ln_mean")
                    nc.vector.tensor_scalar_mul(mean[:], ps_sum[:], 1.0 / D)
                    msq = lnp.tile([1, T], f32, tag="ln_msq", name="ln_msq")
                    nc.vector.tensor_scalar_mul(msq[:], ps_sq[:], 1.0 / D)
                    m2 = lnp.tile([1, T], f32, tag="ln_m2", name="ln_m2")
                    nc.vector.tensor_mul(m2[:], mean[:], mean[:])
                    var = lnp.tile([1, T], f32, tag="ln_var", name="ln_var")
                    nc.vector.tensor_sub(var[:], msq[:], m2[:])
                    sd = lnp.tile([1, T], f32, tag="ln_sd", name="ln_sd")
                    nc.scalar.activation(sd[:], var[:], AF.Sqrt, bias=epsc[:])
                    rstd = lnp.tile([1, T], f32, tag="ln_rstd", name="ln_rstd")
                    nc.vector.reciprocal(rstd[:], sd[:])
                    mrs = lnp.tile([1, T], f32, tag="ln_mrs", name="ln_mrs")
                    nc.vector.tensor_mul(mrs[:], mean[:], rstd[:])
                    rstd_b = lnp.tile([1, T], bf16, tag="ln_rstdb", name="ln_rstdb")
                    nc.vector.tensor_copy(rstd_b[:], rstd[:])
                    mrs_b = lnp.tile([1, T], bf16, tag="ln_mrsb", name="ln_mrsb")
                    nc.vector.tensor_copy(mrs_b[:], mrs[:])
                    ps_R = ps_pool.tile([P, T], f32, tag="ln_Rb", name="ln_Rb")
                    nc.tensor.matmul(ps_R[:], onesrow[0:1, 0:P], rstd_b[:],
                                     start=True, stop=True)
                    ps_M = ps_pool.tile([P, T], f32, tag="ln_Mb", name="ln_Mb")
                    nc.tensor.matmul(ps_M[:], onesrow[0:1, 0:P], mrs_b[:],
                                     start=True, stop=True)
                    R_b = scr.tile([P, T], bf16, tag="ln_Rsb", name="ln_Rsb")
                    nc.scalar.copy(R_b[:], ps_R[:])
                    M_b = scr.tile([P, T], bf16, tag="ln_Msb", name="ln_Msb")
                    nc.scalar.copy(M_b[:], ps_M[:])
                    for k in range(DC):
                        t1 = scr.tile([P, T], bf16, tag="lnt", name="lnt", bufs=4)
                        nc.vector.tensor_mul(t1[:], xin[k][blk][:], R_b[:])
                        o_ = out_pool.tile([P, T], bf16, tag=out_tag, name=out_tag)
                        nc.vector.tensor_sub(o_[:], t1[:], M_b[:])
                        outs[k][blk] = o_
                return outs

            # ================= Phase LN1 (full sequence) =================
            es_hT = ExitStack()
            hp = es_hT.enter_context(tc.tile_pool(name="hpool", bufs=32))
            ph = ExitStack()
            xp = ph.enter_context(tc.tile_pool(name="xpool", bufs=32))
            pln = ph.enter_context(tc.tile_pool(name="ps_ln1", bufs=2, space="PSUM"))
            xin = [[None] * NB for _ in range(DC)]
            for blk in range(NB):
                for k in range(DC):
                    t_ = xp.tile([P, T], bf16, tag="xT", name="xT")
                    nc.sync.dma_start(
                        t_[:], xT_d[P * k : P * (k + 1), T * blk : T * (blk + 1)]
                    )
                    xin[k][blk] = t_
            hT = layer_norm(pln, xin, NB, hp, "hT")
            ph.close()

            # own-token extraction with the g1 fold:
            # ho[k][:, 128*i:...] = g1_k * hT[k][i][:, 0:128]
            ho = [None] * DC
            for k in range(DC):
                ho[k] = hop.tile([P, T], bf16, tag="ho", name="ho")
                for i in range(NT):
                    nc.vector.tensor_scalar_mul(
                        ho[k][:, P * i : P * (i + 1)], hT[k][i][:, 0:P],
                        g1c[:, k : k + 1],
                    )

            # ================= Phase K (full sequence, local) =================
            es_k = ExitStack()
            kfp = es_k.enter_context(tc.tile_pool(name="kfpool", bufs=32, side="right"))
            ph = ExitStack()
            wkp = ph.enter_context(tc.tile_pool(name="wkres", bufs=8))
            pk = ph.enter_context(tc.tile_pool(name="ps_k", bufs=8, space="PSUM"))
            if True:
                wkt = []
                for k in range(DC):
                    wt = wkp.tile([P, D], bf16, tag="wk", name="wk")
                    nc.sync.dma_start(wt[:], wk_d[P * k : P * (k + 1), :])
                    wkt.append(wt)
                K_sb = [[None] * NB for _ in range(DC)]
                for blk in range(NB):
                    psum = [None] * DC
                    for k in range(DC):
                        for m in range(DC):
                            if k == 0:
                                psum[m] = pk.tile([P, T], f32, tag="kp", name="kp")
                            nc.tensor.matmul(
                                psum[m][:], wkt[k][:, P * m : P * (m + 1)],
                                hT[k][blk][:],
                                start=(k == 0), stop=(k == DC - 1),
                            )
                    for m in range(DC):
                        o_ = kfp.tile([P, T], bf16, tag="K", name="K")
                        nc.vector.tensor_scalar_add(o_[:], psum[m][:], bkc[:, m : m + 1])
                        K_sb[m][blk] = o_
            ph.close()
            es_hT.close()

            # ================= Phase Q (own tokens, feature-major) =================
            es_v = ExitStack()
            vp = es_v.enter_context(tc.tile_pool(name="vpool", bufs=4))
            wvp = es_v.enter_context(tc.tile_pool(name="wvres", bufs=8))
            es_q = ExitStack()
            qp = es_q.enter_context(tc.tile_pool(name="qpool", bufs=8))
            ph = ExitStack()
            pq = ph.enter_context(tc.tile_pool(name="ps_qv", bufs=8, space="PSUM"))
            if True:
                psum = [None] * DC
                for k in range(DC):
                    wt = wp.tile([P, D], bf16, tag="w", name="w")
                    nc.sync.dma_start(wt[:], wq_d[P * k : P * (k + 1), :])
                    for m in range(DC):
                        if k == 0:
                            psum[m] = pq.tile([P, T], f32, tag="qv", name="qv")
                        nc.tensor.matmul(
                            psum[m][:], wt[:, P * m : P * (m + 1)], ho[k][:],
                            start=(k == 0), stop=(k == DC - 1),
                        )
                Q = [None] * DC
                for m in range(DC):
                    Q[m] = qp.tile([P, T], bf16, tag="Q", name="Q")
                    nc.vector.tensor_scalar_add(Q[m][:], psum[m][:], bqc[:, m : m + 1])
                # resident wv for the interleaved V projection
                wvt = []
                for k in range(DC):
                    wt = wvp.tile([P, D], bf16, tag="wv", name="wv")
                    nc.sync.dma_start(wt[:], wv_d[P * k : P * (k + 1), :])
                    wvt.append(wt)
            ph.close()

            # ============ Phase ATT: scores interleaved with V/CS/e16 ============
            # Scores are exp-bound on the activation engine; the V projection,
            # V column sums, and the diagonal pass are emitted between head
            # iterations so the tensor engine fills the exp window. PSUM: score
            # tile 4 banks (bufs=1) + V 2 + CS 1 + e16 1 = 8.
            acc = [sp4.tile([P, 16], f32, tag="acc", name="acc") for _ in range(NT)]
            E16 = [None] * NT
            V = [None] * NT
            rdenom_fm = cp.tile([16, T], f32r, tag="rdenom_fm", name="rdenom_fm")
            phs = ExitStack()
            pa3 = phs.enter_context(tc.tile_pool(name="ps_att3", bufs=1, space="PSUM"))
            pvi = phs.enter_context(tc.tile_pool(name="ps_vi", bufs=2, space="PSUM"))
            pcs = phs.enter_context(tc.tile_pool(name="ps_cs", bufs=1, space="PSUM"))
            pa = phs.enter_context(tc.tile_pool(name="ps_att", bufs=1, space="PSUM"))
            if True:
                for t in range(NT):
                    V[t] = vp.tile([P, D], bf16, tag="V", name="V")
                ps_e = pa.tile([16, T], f32, tag="pe", name="pe")

                def emit_v_unit(u):          # u in 0..7: (t, n) V projection
                    t, n = u // 2, u % 2
                    ps = pvi.tile([P, T], f32, tag="vi", name="vi")
                    nc.tensor.matmul(
                        ps[:], onesrow[0:1, 0:P], bvrow[0:1, T * n : T * (n + 1)],
                        start=True, stop=False,
                    )
                    for k in range(DC):
                        nc.tensor.matmul(
                            ps[:], ho[k][:, P * t : P * (t + 1)],
                            wvt[k][:, T * n : T * (n + 1)],
                            start=False, stop=(k == DC - 1),
                        )
                    nc.vector.tensor_copy(V[t][:, T * n : T * (n + 1)], ps[:])

                def emit_cs_unit(i):         # V column sums for own tile i
                    for half in range(2):
                        ps_c = pcs.tile([1, T], f32, tag="cs", name="cs")
                        nc.tensor.matmul(
                            ps_c[:], onescol[:],
                            V[i][:, T * half : T * (half + 1)],
                            start=True, stop=True,
                        )
                        cs_scr = scr.tile([1, T], bf16, tag="cs_scr", name="cs_scr")
                        nc.vector.tensor_copy(cs_scr[:], ps_c[:])
                        nc.sync.dma_start(
                            kv2_in[2 * i + half : 2 * i + half + 1, :], cs_scr[:]
                        )

                def emit_e16_unit(c):        # diagonal pass, chunk c
                    Tt = scr.tile([P, T], bf16, tag="T", name="T")
                    for i in range(NT):
                        nc.vector.tensor_mul(
                            Tt[:, P * i : P * (i + 1)],
                            Q[c][:, P * i : P * (i + 1)],
                            K_sb[c][i][:, 0:P],
                        )
                    nc.tensor.matmul(
                        ps_e[:], H16T[:, 16 * c : 16 * (c + 1)], Tt[:],
                        start=(c == 0), stop=(c == DC - 1),
                    )

                for h in range(H):
                    c2, r0 = h // 2, HD * (h % 2)
                    for i in range(NT):
                        nch = NB - i          # suffix blocks for q-tile i
                        width = T * nch
                        ps_s = pa3.tile([P, S], f32, tag="s", name="s")
                        for kc in range(nch):
                            nc.tensor.matmul(
                                ps_s[:, T * kc : T * (kc + 1)],
                                Q[c2][r0 : r0 + HD, P * i : P * (i + 1)],
                                K_sb[c2][i + kc][r0 : r0 + HD, :],
                                start=True, stop=(kc > 0),
                            )
                            if kc == 0:
                                # boundary block: accumulate the causal mask
                                # (exp's 1/32 scale leaves NEG/32 ~ -3e7)
                                nc.tensor.matmul(
                                    ps_s[:, 0:T], I128r[:], M1[:],
                                    start=False, stop=True,
                                )
                        esc = scr.tile([P, S], bf16, tag="esc", name="esc", bufs=3)
                        nc.scalar.activation(
                            esc[:, 0:width], ps_s[:, 0:width], AF.Exp,
                            scale=1.0 / 32.0,
                            accum_out=acc[i][:, h : h + 1],
                        )
                    # fill work between heads
                    if h < 8:
                        emit_v_unit(h)
                    elif h < 12:
                        emit_cs_unit(h - 8)
                        if h == 11:
                            nc.gpsimd.collective_compute(
                                "AllGather", ALU.bypass,
                                replica_groups=[[0, 1, 2, 3], [4, 5, 6, 7]],
                                ins=[kv2_in[:]], outs=[kv2_out[:]],
                            )
                    else:
                        emit_e16_unit(2 * (h - 12))
                        emit_e16_unit(2 * (h - 12) + 1)
                e16_sb = cp.tile([16, T], f32, tag="e16_sb", name="e16_sb")
                nc.scalar.activation(e16_sb[:], ps_e[:], AF.Exp, scale=1.0 / 32.0)
            phs.close()
            phn = ExitStack()
            ptr2 = phn.enter_context(tc.tile_pool(name="ps_tr2", bufs=2, space="PSUM"))
            if True:
                # E16 transposes + denominators -> reciprocal, feature-major
                for t in range(NT):
                    ps_tr = ptr2.tile([P, 16], f32, tag="tr1", name="tr1")
                    nc.tensor.transpose(
                        ps_tr[:], e16_sb[0:16, P * t : P * (t + 1)],
                        ident[0:16, 0:16],
                    )
                    E16[t] = sp4.tile([P, 16], bf16, tag="E16", name="E16")
                    nc.vector.tensor_copy(E16[t][:], ps_tr[:])
                for t in range(NT):
                    dn = sp4.tile([P, 16], f32, tag="dn", name="dn")
                    nc.vector.tensor_scalar_add(dn[:], acc[t][:], qcount[:, t : t + 1])
                    nc.vector.reciprocal(dn[:], dn[:])
                    ps_t2 = ptr2.tile([16, P], f32, tag="tr2", name="tr2")
                    nc.tensor.transpose(ps_t2[:], dn[:], ident[:])
                    nc.vector.tensor_copy(rdenom_fm[0:16, P * t : P * (t + 1)], ps_t2[:])
            phn.close()

            es_k.close()
            es_q.close()

            # read back per-tile V sums (needed only for phase NUM)
            csum_all = cp.tile([32, T], bf16, tag="csum_all", name="csum_all")
            nc.sync.dma_start(csum_all[:], kv2_out[:])

            # ================= Phase NUM =================
            attn = [None] * DC
            es_h2n = ExitStack()
            h2np = es_h2n.enter_context(tc.tile_pool(name="h2npool", bufs=8, side="right"))
            es_h2 = ExitStack()
            h2p = es_h2.enter_context(tc.tile_pool(name="h2pool", bufs=8, side="right"))
            es_attn = ExitStack()
            ap = es_attn.enter_context(tc.tile_pool(name="attnpool", bufs=8, side="right"))
            ph = ExitStack()
            vdp = ph.enter_context(tc.tile_pool(name="vdpool", bufs=4))
            pn = ph.enter_context(tc.tile_pool(name="ps_num", bufs=5, space="PSUM"))
            prd = ph.enter_context(tc.tile_pool(name="ps_rd", bufs=2, space="PSUM"))
            pp = ph.enter_context(tc.tile_pool(name="ps_p", bufs=1, space="PSUM"))
            if True:
                P_sb = [None] * NT
                for i in range(NT):
                    P_sb[i] = cp.tile([1, D], bf16, tag=f"P_sb{i}", name=f"P_sb{i}")
                    for half in range(2):
                        ps_P = pp.tile([1, T], f32, tag="pP", name="pP")
                        nc.tensor.matmul(
                            ps_P[:], w32[:, 2 * i + half : 2 * i + half + 1],
                            csum_all[:],
                            start=True, stop=True,
                        )
                        nc.vector.tensor_copy(
                            P_sb[i][0:1, T * half : T * (half + 1)], ps_P[:]
                        )
                Vd = [None] * NT
                for t in range(NT):
                    Vd[t] = vdp.tile([P, D], bf16, tag="Vd", name="Vd")
                    nc.vector.tensor_mul(
                        Vd[t][:].rearrange("p (h x) -> p h x", h=16),
                        V[t][:].rearrange("p (h x) -> p h x", h=16),
                        E16[t][:, :, None].broadcast_to([P, 16, HD]),
                    )
                for c in range(DC):
                    ps_n = pn.tile([P, T], f32, tag="n", name="n")
                    for i in range(NT):
                        sl = ps_n[:, P * i : P * (i + 1)]
                        nc.tensor.matmul(
                            sl, P_sb[i][0:1, P * c : P * (c + 1)],
                            onesrow[0:1, 0:P],
                            start=True, stop=False,
                        )
                        nc.tensor.matmul(
                            sl, V[i][:, P * c : P * (c + 1)], L128[:],
                            start=False, stop=False,
                        )
                        nc.tensor.matmul(
                            sl, Vd[i][:, P * c : P * (c + 1)], I128b[:],
                            start=False, stop=True,
                        )
                    ps_r = prd.tile([P, T], f32, tag="rd", name="rd")
                    nc.tensor.matmul(
                        ps_r[:], H16b[:, P * c : P * (c + 1)], rdenom_fm[:],
                        start=True, stop=True,
                    )
                    rd_sb = scr.tile([P, T], f32, tag="rds", name="rds")
                    nc.scalar.copy(rd_sb[:], ps_r[:])
                    attn[c] = ap.tile([P, T], bf16, tag="attn", name="attn")
                    nc.vector.tensor_mul(attn[c][:], ps_n[:], rd_sb[:])

            ph.close()
            es_v.close()

            # ================= Phase WO (+ residual) =================
            h2 = [[None] for _ in range(DC)]
            ph = ExitStack()
            pw = ph.enter_context(tc.tile_pool(name="ps_wo", bufs=8, space="PSUM"))
            if True:
                psum = [None] * DC
                for k in range(DC):
                    wt = wp.tile([P, D], bf16, tag="w", name="w")
                    nc.sync.dma_start(wt[:], wo_d[P * k : P * (k + 1), :])
                    for m in range(DC):
                        if k == 0:
                            psum[m] = pw.tile([P, T], f32, tag="wo", name="wo")
                        nc.tensor.matmul(
                            psum[m][:], wt[:, P * m : P * (m + 1)], attn[k][:],
                            start=(k == 0), stop=(k == DC - 1),
                        )
                for m in range(DC):
                    t_ = h2p.tile([P, T], bf16, tag="h2", name="h2")
                    nc.vector.scalar_tensor_tensor(
                        t_[:], psum[m][:], boc[:, m : m + 1],
                        ho[m][:], ALU.add, ALU.add,
                    )
                    h2[m][0] = t_

            ph.close()
            es_attn.close()

            # ================= Phase LN2 =================
            ph = ExitStack()
            pln2 = ph.enter_context(tc.tile_pool(name="ps_ln2", bufs=1, space="PSUM"))
            h2n = layer_norm(pln2, h2, 1, h2np, "h2n")
            ph.close()
            es_h2.close()

            # ================= Phase FFN1 =================
            a1 = [None] * (DFF // P)
            ph = ExitStack()
            es_a1 = ExitStack()
            a1p = es_a1.enter_context(tc.tile_pool(name="a1pool", bufs=32))
            pf1 = ph.enter_context(tc.tile_pool(name="ps_f1", bufs=8, space="PSUM"))
            if True:
                for g in range(DFF // P // DC):
                    psum = [None] * DC
                    for k in range(DC):
                        wt = wp.tile([P, D], bf16, tag="w", name="w")
                        nc.sync.dma_start(
                            wt[:], w1_d[P * k : P * (k + 1), D * g : D * (g + 1)]
                        )
                        for m in range(DC):
                            if k == 0:
                                psum[m] = pf1.tile([P, T], f32, tag="f1", name="f1")
                            nc.tensor.matmul(
                                psum[m][:], wt[:, P * m : P * (m + 1)], h2n[k][0][:],
                                start=(k == 0), stop=(k == DC - 1),
                            )
                    for m in range(DC):
                        idx = DC * g + m
                        a1[idx] = a1p.tile([P, T], bf16, tag="a1", name="a1")
                        nc.vector.tensor_scalar(
                            a1[idx][:], psum[m][:], b1c[:, idx : idx + 1], 0.0,
                            ALU.add, ALU.max,
                        )

            ph.close()
            es_h2n.close()
            es_ho.close()

            # ================= Phase FFN2 =================
            ph = ExitStack()
            op = ph.enter_context(tc.tile_pool(name="opool", bufs=8))
            pf2 = ph.enter_context(tc.tile_pool(name="ps_f2", bufs=8, space="PSUM"))
            if True:
                psum = [None] * DC
                for k in range(DFF // P):
                    wt = wp.tile([P, D], bf16, tag="w", name="w")
                    nc.sync.dma_start(wt[:], w2_d[P * k : P * (k + 1), :])
                    for m in range(DC):
                        if k == 0:
                            psum[m] = pf2.tile([P, T], f32, tag="f2", name="f2")
                        nc.tensor.matmul(
                            psum[m][:], wt[:, P * m : P * (m + 1)], a1[k][:],
                            start=(k == 0), stop=(k == DFF // P - 1),
                        )
                for m in range(DC):
                    o_ = op.tile([P, T], f32, tag="o", name="o")
                    nc.vector.tensor_scalar(
                        o_[:], psum[m][:], b2c[:, m : m + 1], 0.0,
                        ALU.add, ALU.max,
                    )
                    nc.sync.dma_start(out_d[P * m : P * (m + 1), :], o_[:])
            ph.close()
            es_a1.close()

    return nc


def _host_inputs(x, g1, be1, wq, bq, wk, bk, wv, bv, wo, bo, g2, be2,
                 w1, b1, w2, b2):
    f = np.float32
    bf = ml_dtypes.bfloat16
    x = np.asarray(x, f)

    def cols(v, n):
        return np.ascontiguousarray(np.asarray(v, f).reshape(n, P).T)

    g1 = np.asarray(g1, f); be1 = np.asarray(be1, f)
    g2 = np.asarray(g2, f); be2 = np.asarray(be2, f)
    wq = np.asarray(wq, f); wk = np.asarray(wk, f); wv = np.asarray(wv, f)
    wo = np.asarray(wo, f); w1 = np.asarray(w1, f); w2 = np.asarray(w2, f)
    # gamma/beta folds: the kernel's LN emits plain z = (x - m) * rstd.
    # h1 = g1*z1 + be1 reaches Q/V through ho = g1*z1 (bias be1@w folded into
    # bq/bv), reaches K through wk' = g1*wk (bias be1@wk folded into bk), and
    # reaches the residual via ho + bo' with bo' = bo + be1. h2n = g2*z2 + be2
    # reaches FFN1 through w1' = g2*w1 and b1' = b1 + be2@w1.
    shared = {
        "wq": wq.astype(bf), "wk": (g1[:, None] * wk).astype(bf),
        "wv": wv.astype(bf), "wo": wo.astype(bf),
        "w1": (g2[:, None] * w1).astype(bf), "w2": w2.astype(bf),
        "g1c": cols(g1, DC),
        "bqc": cols(np.asarray(bq, f) + be1 @ wq, DC),
        "bkc": cols(np.asarray(bk, f) + be1 @ wk, DC),
        "boc": cols(np.asarray(bo, f) + be1, DC),
        "b1c": cols(np.asarray(b1, f) + be2 @ w1, DFF // P),
        "b2c": cols(b2, DC),
        "bvrow": (np.asarray(bv, f).reshape(1, D)
                  + (be1 @ wv).reshape(1, D)).astype(bf),
        "L128": np.triu(np.ones((P, P), f), 1).astype(bf),
        "I128b": np.eye(P, dtype=f).astype(bf),
        "I128r": np.eye(P, dtype=f),
        "ident": np.eye(P, dtype=f),
        "onesrow": np.ones((1, S), f).astype(bf),
        "onescol": np.ones((P, 1), f).astype(bf),
    }
    H16T = np.zeros((P, P), f)
    H16b = np.zeros((16, D), f)
    for c in range(DC):
        for i in range(2):
            h = 2 * c + i
            H16T[HD * i : HD * (i + 1), 16 * c + h] = 1.0
            H16b[h, P * c + HD * i : P * c + HD * (i + 1)] = 1.0
    shared["H16T"] = H16T.astype(bf)
    shared["H16b"] = H16b

    in_maps = []
    for core in range(NCORES):
        b, j = core // CH, core % CH
        m = dict(shared)
        # full batch, feature-major, each 512-block rotated so the core's own
        # 128-tile sits at position 0: block s order = tiles [4s + (j+r)%4]
        xb = x[b]                                     # [S, D]
        blocks = []
        for s_ in range(NB):
            tiles = [xb[P * (CH * s_ + (j + r) % CH) : P * (CH * s_ + (j + r) % CH + 1), :]
                     for r in range(CH)]
            blocks.append(np.concatenate(tiles, axis=0))
        xperm = np.concatenate(blocks, axis=0)        # [S, D] permuted
        m["xT"] = np.ascontiguousarray(xperm.T).astype(bf)
        # qcount: global row index of own tile i, row p
        qc = np.stack(
            [P * (j + CH * i) + np.arange(P, dtype=f) for i in range(NT)], axis=1
        )
        m["qcount"] = np.ascontiguousarray(qc)
        # boundary-block mask in rotated coordinates: position c holds tile
        # r(c) = (j + c//128) % 4; keep iff 128*r(c) + (c%128) >= 128j + p
        c_ = np.arange(T)[None, :]
        p_ = np.arange(P)[:, None]
        rposc = (j + c_ // P) % CH
        keep = (P * rposc + (c_ % P)) >= (P * j + p_)
        m["M1"] = np.where(keep, 0.0, NEG).astype(f)
        # prefix weights: P_i sums vtsum over global tiles g' < j + 4*i,
        # AG row layout: rank r rows [8r+2i'+h'] = (g'=r+4i', half h')
        w32 = np.zeros((32, 8), f)
        for i in range(NT):
            for h_ in range(2):
                for r in range(CH):
                    for i2 in range(NT):
                        if r + CH * i2 < j + CH * i:
                            w32[8 * r + 2 * i2 + h_, 2 * i + h_] = 1.0
        m["w32"] = w32.astype(bf)
        in_maps.append(m)
    return in_maps


_nc_cache = None


def kernel(**inputs):
    global _nc_cache
    if _nc_cache is None:
        _nc_cache = _build()
    nc = _nc_cache
    in_maps = _host_inputs(**inputs)
    res = run_bass_kernel_spmd(nc, in_maps, list(range(NCORES)))
    out = np.empty((B, S, D), np.float32)
    for core in range(NCORES):
        b, j = core // CH, core % CH
        oT = res.results[core]["outT"]
        for i in range(NT):
            g = j + CH * i
            out[b, P * g : P * (g + 1), :] = oT[:, P * i : P * (i + 1)].T
    return out


def make_timed_runner(**inputs):
    """Build the program once and return (run_fn, assemble_fn) where run_fn()
    executes on the 8 cores re-using the compiled NEFF (for timing loops)."""
    import jax
    from jax.sharding import Mesh, PartitionSpec
    from jax.experimental.shard_map import shard_map
    from concourse import bass2jax

    global _nc_cache
    if _nc_cache is None:
        _nc_cache = _build()
    nc = _nc_cache
    in_maps = _host_inputs(**inputs)

    bass2jax.install_neuronx_cc_hook()
    partition_name = nc.partition_id_tensor.name if nc.partition_id_tensor else None
    in_names, out_names, out_avals, zero_outs = [], [], [], []
    for alloc in nc.m.functions[0].allocations:
        if not isinstance(alloc, mybir.MemoryLocationSet):
            continue
        name = alloc.memorylocations[0].name
        if alloc.kind == "ExternalInput":
            if name != partition_name:
                in_names.append(name)
        elif alloc.kind == "ExternalOutput":
            out_names.append(name)
            shape = tuple(alloc.tensor_shape)
            dtype = mybir.dt.np(alloc.dtype)
            out_avals.append(jax.core.ShapedArray(shape, dtype))
            zero_outs.append(np.zeros(shape, dtype))
    n_params = len(in_names)
    all_in = in_names + out_names
    if partition_name is not None:
        all_in.append(partition_name)

    def _body(*args):
        operands = list(args)
        if partition_name is not None:
            operands.append(bass2jax.partition_id_tensor())
        outs = bass2jax._bass_exec_p.bind(
            *operands,
            out_avals=tuple(out_avals),
            in_names=tuple(all_in[: n_params + len(out_names) + (0 if partition_name is None else 1)]),
            out_names=tuple(out_names),
            lowering_input_output_aliases=(),
            sim_require_finite=True,
            sim_require_nnan=True,
            nc=nc,
        )
        return tuple(outs)

    devices = jax.devices()[:NCORES]
    mesh = Mesh(np.asarray(devices), ("core",))
    nin = n_params + len(out_names)
    sharded = jax.jit(
        shard_map(
            _body, mesh=mesh,
            in_specs=(PartitionSpec("core"),) * nin,
            out_specs=(PartitionSpec("core"),) * len(out_names),
            check_rep=False,
        ),
        keep_unused=True,
    )
    concat_in = [
        np.concatenate([np.asarray(in_maps[c][nm]) for c in range(NCORES)], axis=0)
        for nm in in_names
    ]
    concat_zeros = [
        np.zeros((NCORES * z.shape[0], *z.shape[1:]), z.dtype) for z in zero_outs
    ]
    from jax.sharding import NamedSharding
    sh = NamedSharding(mesh, PartitionSpec("core"))
    args = [jax.device_put(a, sh) for a in concat_in + concat_zeros]

    def run():
        outs = sharded(*args)
        jax.block_until_ready(outs)
        return outs

    def run_async():
        return sharded(*args)

    def assemble(outs):
        res = np.asarray(outs[out_names.index("outT")]).reshape(NCORES, D, T)
        out = np.empty((B, S, D), np.float32)
        for core in range(NCORES):
            b, j = core // CH, core % CH
            for i in range(NT):
                g = j + CH * i
                out[b, P * g : P * (g + 1), :] = res[core][:, P * i : P * (i + 1)].T
        return out

    run.run_async = run_async
    return run, assemble


# revision 26
# speedup vs baseline: 1.0560x; 1.0560x over previous
"""Trainium2 Bass kernel for nn_DecoderBlock (B=2, S=2048, D=1024, DFF=4096, H=16).

Sharding: 8 cores = 2 batches x 4 cores. Each core owns 512 interleaved
128-token q-tiles (core j of a batch owns global tiles j, j+4, j+8, j+12) but
receives the FULL batch sequence (feature-major, bf16) with each 512-token
block rotated so the core's own 128-token tile sits at position 0 of every
block. LayerNorm1 and the K projection are computed redundantly for the full
sequence on every core, which removes the K AllGather entirely (the cost
model charges 15us fixed latency per collective; four sliced 1MB gathers
serialized at ~41us each on the collective engine in the baseline). The only
remaining collective is the tiny per-tile V-column-sum AllGather (32KB).

The reference module's triu/transpose softmax degenerates mathematically:
    c[q,k] = 1/denom(q)            for k < q
           = exp(s[q,q])/denom(q)  for k == q
           = 0                     for k > q
    denom(q) = q + sum_{k>=q} exp(s[q,k])
so attention output = (prefix_sum(V) + exp(diag)*V_own) / denom, and V never
crosses cores - only per-128-row-tile V column sums.

Numerics: all big matmuls run in bf16 (weights shipped bf16 from host), PSUM
accumulation stays f32, LayerNorm statistics stay f32. The small [*,128]-free
matmuls of the numerator phase run bf16 (f32r below 256 free columns costs 4
cycles/row on TRN2). Score exp row-sums use the activation engine's fused
accumulator instead of DVE reductions.

LayerNorm gamma/beta are folded into the downstream weights on the host
(wk' = g1*wk, bk' = bk + be1@wk, etc.; the residual picks up g1 via the
own-token extraction and be1 via bo' = bo + be1), so the on-chip LN applies
only (x - mean) * rstd (two DVE/Pool passes). The V projection runs with
resident wv in a 2-PSUM-bank footprint and is emission-interleaved with the
score loop (together with V column sums and the diagonal pass) so the tensor
engine fills the activation-bound exp window.
"""
import sys

sys.path.insert(0, "/opt/trn_rl_repo")

import ml_dtypes
import numpy as np

from contextlib import ExitStack

import concourse.bass as bass
import concourse.mybir as mybir
import concourse.tile as tile
from concourse.bass_utils import run_bass_kernel_spmd
from concourse.vector_clock import ScopedClock

# ---------------------------------------------------------------------------
# Patch for this walrus build: it rejects more than one sync-wait command per
# instruction. Split multi-wait instructions into preceding same-engine NOPs
# (program order on the engine preserves semantics), both for scheduled
# instructions and for the TileContext tail drain.
# ---------------------------------------------------------------------------
_MAX_WAITS = 1
_orig_lower = tile.TileContext._lower_ordered_insts


def _split_waits(ordered):
    for bb_name, insts in ordered.items():
        new_insts = []
        for inst in insts:
            si = inst.sync_info
            if si is not None and si.on_wait and len(si.on_wait) > _MAX_WAITS:
                waits = list(si.on_wait)
                for i, w in enumerate(waits[:-_MAX_WAITS]):
                    new_insts.append(
                        mybir.InstNoOp(
                            name=f"{inst.name}-ws{i}",
                            sync_info=mybir.SyncInfo(on_wait=[w], on_update=[]),
                            bass_nofuse=True,
                            engine=inst.engine,
                        )
                    )
                inst.sync_info = mybir.SyncInfo(
                    on_wait=waits[-_MAX_WAITS:],
                    on_update=list(si.on_update) if si.on_update else [],
                )
            new_insts.append(inst)
        ordered[bb_name] = new_insts
    return ordered


def _lower_ordered_insts(self, ordered):
    return _orig_lower(self, _split_waits(ordered))


def _drain_and_barrier(self, tick_clock, wait_clock):
    nc = self.nc
    drain_inst = nc.sync.drain()
    wait_clock.add_sem_waits(
        drain_inst.ins, ScopedClock({None: tick_clock.global_clock})
    )
    si = drain_inst.ins.sync_info
    waits = list(si.on_wait) if si is not None else []
    if len(waits) > _MAX_WAITS:
        drain_inst.ins.sync_info = mybir.SyncInfo(
            on_wait=waits[:_MAX_WAITS],
            on_update=list(si.on_update) if si.on_update else [],
        )
        for i in range(_MAX_WAITS, len(waits), _MAX_WAITS):
            nop = nc.sync.nop(nofuse=True)
            nop.ins.sync_info = mybir.SyncInfo(
                on_wait=waits[i : i + _MAX_WAITS], on_update=[]
            )
    nc.all_engine_barrier()
    assert self.sems is not None
    popped = nc._tile_sem_poison_stack.pop()
    assert popped is self._sem_poison
    nc.clear_and_free_semaphores(list(self.sems.allocated().values()))
    nc.all_engine_barrier()


tile.TileContext._lower_ordered_insts = _lower_ordered_insts
tile.TileContext._drain_and_barrier = _drain_and_barrier

# ---------------------------------------------------------------------------

B, S, D, DFF, H = 2, 2048, 1024, 4096, 16
HD = D // H          # 64
EPS = 1e-5
NCORES = 8
CH = 4               # cores per batch / 128-tiles per 512-block
T = S // CH          # 512 own tokens per core
P = 128
NT = T // P          # 4 own q-tiles per core
NB = S // T          # 4 global 512-token blocks
DC = D // P          # 8 feature chunks

f32 = mybir.dt.float32
f32r = mybir.dt.float32r
bf16 = mybir.dt.bfloat16
AF = mybir.ActivationFunctionType
ALU = mybir.AluOpType
AX = mybir.AxisListType
NEG = -1.0e9


def _build():
    nc = bass.Bass(num_devices=NCORES)

    def par(name, shape, dt):
        return nc.declare_dram_parameter(name, shape, dt, isOutput=False)

    # per-core data
    xT_d = par("xT", [D, S], bf16)                # full batch, block-rotated
    qcount_d = par("qcount", [P, NT], f32)        # col i = global row index of tile i
    M1_d = par("M1", [P, T], f32r)                # boundary-block additive mask
    w32_d = par("w32", [32, 8], bf16)             # prefix tile-sum weights
    # shared weights (natural [din, dout] layout = lhsT), bf16
    wq_d = par("wq", [D, D], bf16)
    wk_d = par("wk", [D, D], bf16)
    wv_d = par("wv", [D, D], bf16)
    wo_d = par("wo", [D, D], bf16)
    w1_d = par("w1", [D, DFF], bf16)
    w2_d = par("w2", [DFF, D], bf16)
    # per-partition gamma/bias columns ([P, n_chunks], f32)
    g1_d = par("g1c", [P, DC], f32)
    bq_d = par("bqc", [P, DC], f32)
    bk_d = par("bkc", [P, DC], f32)
    bo_d = par("boc", [P, DC], f32)
    b1_d = par("b1c", [P, DFF // P], f32)
    b2_d = par("b2c", [P, DC], f32)
    bv_d = par("bvrow", [1, D], bf16)             # V bias as a row (free-dim)
    # shared constant matrices
    L128_d = par("L128", [P, P], bf16)            # L[k,q] = 1 if k < q
    I128b_d = par("I128b", [P, P], bf16)          # identity (bf16, NUM phase)
    I128r_d = par("I128r", [P, P], f32r)          # identity (f32r, mask matmul)
    ident_d = par("ident", [P, P], f32)
    H16T_d = par("H16T", [P, P], bf16)            # [:,16c+h]: head-h rows in chunk c
    H16b_d = par("H16b", [16, D], f32r)           # [h,128c+p]: head(p of chunk c)==h
    onesrow_d = par("onesrow", [1, S], bf16)
    onescol_d = par("onescol", [P, 1], bf16)

    out_d = nc.declare_dram_parameter("outT", [D, T], f32, isOutput=True)

    kv2_in = nc.dram_tensor("kv2_in", [8, T], bf16)
    kv2_out = nc.dram_tensor("kv2_out", [8 * CH, T], bf16)

    with tile.TileContext(nc, pool_alloc_mode="queue") as tc, ExitStack() as es:
            cp = es.enter_context(tc.tile_pool(name="cpool", bufs=1))
            lnp = es.enter_context(tc.tile_pool(name="lnstat", bufs=1))
            sp4 = es.enter_context(tc.tile_pool(name="small4", bufs=4))
            scr = es.enter_context(tc.tile_pool(name="scr", bufs=2))
            wp = es.enter_context(tc.tile_pool(name="wstream", bufs=8))
            es_ho = ExitStack()
            hop = es_ho.enter_context(tc.tile_pool(name="hopool", bufs=8, side="right"))

            # ---- load constants ----
            def load(pool, name, src, shape, dt, tag=None):
                t_ = pool.tile(shape, dt, tag=tag or name, name=tag or name)
                nc.sync.dma_start(t_[:], src[:])
                return t_

            qcount = load(cp, "qcount", qcount_d, [P, NT], f32)
            M1 = load(cp, "M1", M1_d, [P, T], f32r)
            w32 = load(cp, "w32", w32_d, [32, 8], bf16)
            g1c = load(cp, "g1c", g1_d, [P, DC], f32)
            bqc = load(cp, "bqc", bq_d, [P, DC], f32)
            bkc = load(cp, "bkc", bk_d, [P, DC], f32)
            boc = load(cp, "boc", bo_d, [P, DC], f32)
            b1c = load(cp, "b1c", b1_d, [P, DFF // P], f32)
            b2c = load(cp, "b2c", b2_d, [P, DC], f32)
            bvrow = load(cp, "bvrow", bv_d, [1, D], bf16)
            ident = load(cp, "ident", ident_d, [P, P], f32)
            H16T = load(cp, "H16T", H16T_d, [P, P], bf16)
            H16b = load(cp, "H16b", H16b_d, [16, D], f32r)
            onesrow = load(cp, "onesrow", onesrow_d, [1, S], bf16)
            onescol = load(cp, "onescol", onescol_d, [P, 1], bf16)
            L128 = load(cp, "L128", L128_d, [P, P], bf16)
            I128b = load(cp, "I128b", I128b_d, [P, P], bf16)
            I128r = load(cp, "I128r", I128r_d, [P, P], f32r)
            epsc = cp.tile([1, 1], f32, tag="epsc", name="epsc")
            nc.vector.memset(epsc[:], EPS)

            # ---- LayerNorm for ONE 512-token block (feature-major) ----
            # Produces plain (x - mean) * rstd; gamma/beta are folded into the
            # downstream weights on the host.
            def layer_norm_block(ps_pool, xblk, out_pool, out_tag):
                sq = []
                for k in range(DC):
                    s_ = scr.tile([P, T], bf16, tag="lnsq", name="lnsq", bufs=8)
                    nc.scalar.activation(s_[:], xblk[k][:], AF.Square)
                    sq.append(s_)
                ps_sum = ps_pool.tile([1, T], f32, tag="ln_sum", name="ln_sum")
                for k in range(DC):
                    nc.tensor.matmul(ps_sum[:], onescol[:], xblk[k][:],
                                     start=(k == 0), stop=(k == DC - 1))
                ps_sq = ps_pool.tile([1, T], f32, tag="ln_sq", name="ln_sq")
                for k in range(DC):
                    nc.tensor.matmul(ps_sq[:], onescol[:], sq[k][:],
                                     start=(k == 0), stop=(k == DC - 1))
                mean = lnp.tile([1, T], f32, tag="ln_mean", name="ln_mean")
                nc.vector.tensor_scalar_mul(mean[:], ps_sum[:], 1.0 / D)
                msq = lnp.tile([1, T], f32, tag="ln_msq", name="ln_msq")
                nc.vector.tensor_scalar_mul(msq[:], ps_sq[:], 1.0 / D)
                m2 = lnp.tile([1, T], f32, tag="ln_m2", name="ln_m2")
                nc.vector.tensor_mul(m2[:], mean[:], mean[:])
                var = lnp.tile([1, T], f32, tag="ln_var", name="ln_var")
                nc.vector.tensor_sub(var[:], msq[:], m2[:])
                sd = lnp.tile([1, T], f32, tag="ln_sd", name="ln_sd")
                nc.scalar.activation(sd[:], var[:], AF.Sqrt, bias=epsc[:])
                rstd = lnp.tile([1, T], f32, tag="ln_rstd", name="ln_rstd")
                nc.vector.reciprocal(rstd[:], sd[:])
                mrs = lnp.tile([1, T], f32, tag="ln_mrs", name="ln_mrs")
                nc.vector.tensor_mul(mrs[:], mean[:], rstd[:])
                rstd_b = lnp.tile([1, T], bf16, tag="ln_rstdb", name="ln_rstdb")
                nc.vector.tensor_copy(rstd_b[:], rstd[:])
                mrs_b = lnp.tile([1, T], bf16, tag="ln_mrsb", name="ln_mrsb")
                nc.vector.tensor_copy(mrs_b[:], mrs[:])
                ps_R = ps_pool.tile([P, T], f32, tag="ln_Rb", name="ln_Rb")
                nc.tensor.matmul(ps_R[:], onesrow[0:1, 0:P], rstd_b[:],
                                 start=True, stop=True)
                ps_M = ps_pool.tile([P, T], f32, tag="ln_Mb", name="ln_Mb")
                nc.tensor.matmul(ps_M[:], onesrow[0:1, 0:P], mrs_b[:],
                                 start=True, stop=True)
                R_b = scr.tile([P, T], bf16, tag="ln_Rsb", name="ln_Rsb")
                nc.scalar.copy(R_b[:], ps_R[:])
                M_b = scr.tile([P, T], bf16, tag="ln_Msb", name="ln_Msb")
                nc.scalar.copy(M_b[:], ps_M[:])
                outs = []
                for k in range(DC):
                    t1 = scr.tile([P, T], bf16, tag="lnt", name="lnt", bufs=2)
                    nc.vector.tensor_mul(t1[:], xblk[k][:], R_b[:])
                    o_ = out_pool.tile([P, T], bf16, tag=out_tag, name=out_tag)
                    nc.vector.tensor_sub(o_[:], t1[:], M_b[:])
                    outs.append(o_)
                return outs

            # ========= Phase LN1 + K, fused per 512-token block =========
            # LN of block b+1 (DVE/Act) overlaps the K projection of block b
            # (PE). PSUM: LN stats/broadcast 4 banks (bufs=1) + K 4 banks.
            es_hT = ExitStack()
            hp = es_hT.enter_context(tc.tile_pool(name="hpool", bufs=32))
            es_k = ExitStack()
            kfp = es_k.enter_context(tc.tile_pool(name="kfpool", bufs=32, side="right"))
            ph = ExitStack()
            xp = ph.enter_context(tc.tile_pool(name="xpool", bufs=12))
            wkp = ph.enter_context(tc.tile_pool(name="wkres", bufs=8))
            pln = ph.enter_context(tc.tile_pool(name="ps_ln1", bufs=1, space="PSUM"))
            pk = ph.enter_context(tc.tile_pool(name="ps_k", bufs=4, space="PSUM"))
            wkt = []
            for k in range(DC):
                wt = wkp.tile([P, D], bf16, tag="wk", name="wk")
                nc.sync.dma_start(wt[:], wk_d[P * k : P * (k + 1), :])
                wkt.append(wt)
            xin = [[None] * NB for _ in range(DC)]
            for blk in range(NB):
                for k in range(DC):
                    t_ = xp.tile([P, T], bf16, tag="xT", name="xT")
                    nc.sync.dma_start(
                        t_[:], xT_d[P * k : P * (k + 1), T * blk : T * (blk + 1)]
                    )
                    xin[k][blk] = t_
            hT = [[None] * NB for _ in range(DC)]
            K_sb = [[None] * NB for _ in range(DC)]
            ho = [None] * DC
            for k in range(DC):
                ho[k] = hop.tile([P, T], bf16, tag="ho", name="ho")
            for blk in range(NB):
                houts = layer_norm_block(pln, [xin[k][blk] for k in range(DC)],
                                         hp, "hT")
                for k in range(DC):
                    hT[k][blk] = houts[k]
                    # own-token extraction with the g1 fold (own tile = block
                    # position 0): ho[k][:, 128*blk:] = g1_k * hT[k][blk][:, 0:128]
                    nc.vector.tensor_scalar_mul(
                        ho[k][:, P * blk : P * (blk + 1)], houts[k][:, 0:P],
                        g1c[:, k : k + 1],
                    )
                for half in range(2):
                    psum = [None] * 4
                    for k in range(DC):
                        for mi in range(4):
                            m = 4 * half + mi
                            if k == 0:
                                psum[mi] = pk.tile([P, T], f32, tag="kp", name="kp")
                            nc.tensor.matmul(
                                psum[mi][:], wkt[k][:, P * m : P * (m + 1)],
                                hT[k][blk][:],
                                start=(k == 0), stop=(k == DC - 1),
                            )
                    for mi in range(4):
                        m = 4 * half + mi
                        o_ = kfp.tile([P, T], bf16, tag="K", name="K")
                        nc.vector.tensor_scalar_add(o_[:], psum[mi][:], bkc[:, m : m + 1])
                        K_sb[m][blk] = o_
            ph.close()
            es_hT.close()

            # ================= Phase Q (own tokens, feature-major) =================
            es_v = ExitStack()
            vp = es_v.enter_context(tc.tile_pool(name="vpool", bufs=4))
            wvp = es_v.enter_context(tc.tile_pool(name="wvres", bufs=8))
            es_q = ExitStack()
            qp = es_q.enter_context(tc.tile_pool(name="qpool", bufs=8))
            ph = ExitStack()
            pq = ph.enter_context(tc.tile_pool(name="ps_qv", bufs=8, space="PSUM"))
            if True:
                psum = [None] * DC
                for k in range(DC):
                    wt = wp.tile([P, D], bf16, tag="w", name="w")
                    nc.sync.dma_start(wt[:], wq_d[P * k : P * (k + 1), :])
                    for m in range(DC):
                        if k == 0:
                            psum[m] = pq.tile([P, T], f32, tag="qv", name="qv")
                        nc.tensor.matmul(
                            psum[m][:], wt[:, P * m : P * (m + 1)], ho[k][:],
                            start=(k == 0), stop=(k == DC - 1),
                        )
                Q = [None] * DC
                for m in range(DC):
                    Q[m] = qp.tile([P, T], bf16, tag="Q", name="Q")
                    nc.vector.tensor_scalar_add(Q[m][:], psum[m][:], bqc[:, m : m + 1])
                # resident wv for the interleaved V projection
                wvt = []
                for k in range(DC):
                    wt = wvp.tile([P, D], bf16, tag="wv", name="wv")
                    nc.sync.dma_start(wt[:], wv_d[P * k : P * (k + 1), :])
                    wvt.append(wt)
            ph.close()

            # ============ Phase ATT: scores interleaved with V ============
            # Scores are exp-bound on the activation engine. Each (head,
            # q-tile) suffix is split into <=1024-col psum units so three can
            # be in flight (6 banks) while the V projection (2 banks) fills
            # the tensor engine between heads.
            acc2 = [sp4.tile([P, 2 * H], f32, tag="acc2", name="acc2")
                    for _ in range(NT)]
            for t in range(NT):
                nc.vector.memset(acc2[t][:], 0.0)
            E16 = [None] * NT
            V = [None] * NT
            rdenom_fm = cp.tile([16, T], f32r, tag="rdenom_fm", name="rdenom_fm")
            # (start, end) column ranges of the psum-local suffix per q-tile
            SEGS = [[(0, 1024), (1024, 2048)], [(0, 1024), (1024, 1536)],
                    [(0, 1024)], [(0, 512)]]
            phs = ExitStack()
            pa3 = phs.enter_context(tc.tile_pool(name="ps_att3", bufs=3, space="PSUM"))
            pvi = phs.enter_context(tc.tile_pool(name="ps_vi", bufs=2, space="PSUM"))
            if True:
                for t in range(NT):
                    V[t] = vp.tile([P, D], bf16, tag="V", name="V")

                def emit_v_unit(u):          # u in 0..7: (t, n) V projection
                    t, n = u // 2, u % 2
                    ps = pvi.tile([P, T], f32, tag="vi", name="vi")
                    nc.tensor.matmul(
                        ps[:], onesrow[0:1, 0:P], bvrow[0:1, T * n : T * (n + 1)],
                        start=True, stop=False,
                    )
                    for k in range(DC):
                        nc.tensor.matmul(
                            ps[:], ho[k][:, P * t : P * (t + 1)],
                            wvt[k][:, T * n : T * (n + 1)],
                            start=False, stop=(k == DC - 1),
                        )
                    nc.vector.tensor_copy(V[t][:, T * n : T * (n + 1)], ps[:])

                for h in range(H):
                    c2, r0 = h // 2, HD * (h % 2)
                    for i in range(NT):
                        for sidx, (c0, c1) in enumerate(SEGS[i]):
                            ps_s = pa3.tile([P, 1024], f32, tag="s", name="s")
                            for kc in range(c0 // T, (c1 + T - 1) // T):
                                nc.tensor.matmul(
                                    ps_s[:, T * kc - c0 : T * (kc + 1) - c0],
                                    Q[c2][r0 : r0 + HD, P * i : P * (i + 1)],
                                    K_sb[c2][i + kc][r0 : r0 + HD, :],
                                    start=True, stop=(kc > 0),
                                )
                                if kc == 0:
                                    # boundary block: accumulate the causal
                                    # mask (exp's 1/32 scale -> NEG/32 ~ -3e7)
                                    nc.tensor.matmul(
                                        ps_s[:, 0:T], I128r[:], M1[:],
                                        start=False, stop=True,
                                    )
                            esc = scr.tile([P, 1024], bf16, tag="esc", name="esc",
                                           bufs=3)
                            nc.scalar.activation(
                                esc[:, 0 : c1 - c0], ps_s[:, 0 : c1 - c0],
                                AF.Exp, scale=1.0 / 32.0,
                                accum_out=acc2[i][:, 2 * h + sidx : 2 * h + sidx + 1],
                            )
                    if h % 2 == 1:
                        emit_v_unit(h // 2)
            phs.close()

            # ---- V column sums + AG, diagonal pass, denominators ----
            phn = ExitStack()
            pcs = phn.enter_context(tc.tile_pool(name="ps_cs", bufs=1, space="PSUM"))
            pa = phn.enter_context(tc.tile_pool(name="ps_att", bufs=1, space="PSUM"))
            ptr2 = phn.enter_context(tc.tile_pool(name="ps_tr2", bufs=2, space="PSUM"))
            if True:
                for i in range(NT):
                    for half in range(2):
                        ps_c = pcs.tile([1, T], f32, tag="cs", name="cs")
                        nc.tensor.matmul(
                            ps_c[:], onescol[:],
                            V[i][:, T * half : T * (half + 1)],
                            start=True, stop=True,
                        )
                        cs_scr = scr.tile([1, T], bf16, tag="cs_scr", name="cs_scr")
                        nc.vector.tensor_copy(cs_scr[:], ps_c[:])
                        nc.sync.dma_start(
                            kv2_in[2 * i + half : 2 * i + half + 1, :], cs_scr[:]
                        )
                nc.gpsimd.collective_compute(
                    "AllGather", ALU.bypass,
                    replica_groups=[[0, 1, 2, 3], [4, 5, 6, 7]],
                    ins=[kv2_in[:]], outs=[kv2_out[:]],
                )
                # diagonal pass e16 = exp(diag/32)
                ps_e = pa.tile([16, T], f32, tag="pe", name="pe")
                for c in range(DC):
                    Tt = scr.tile([P, T], bf16, tag="T", name="T")
                    for i in range(NT):
                        nc.vector.tensor_mul(
                            Tt[:, P * i : P * (i + 1)],
                            Q[c][:, P * i : P * (i + 1)],
                            K_sb[c][i][:, 0:P],
                        )
                    nc.tensor.matmul(
                        ps_e[:], H16T[:, 16 * c : 16 * (c + 1)], Tt[:],
                        start=(c == 0), stop=(c == DC - 1),
                    )
                e16_sb = cp.tile([16, T], f32, tag="e16_sb", name="e16_sb")
                nc.scalar.activation(e16_sb[:], ps_e[:], AF.Exp, scale=1.0 / 32.0)
                for t in range(NT):
                    ps_tr = ptr2.tile([P, 16], f32, tag="tr1", name="tr1")
                    nc.tensor.transpose(
                        ps_tr[:], e16_sb[0:16, P * t : P * (t + 1)],
                        ident[0:16, 0:16],
                    )
                    E16[t] = sp4.tile([P, 16], bf16, tag="E16", name="E16")
                    nc.vector.tensor_copy(E16[t][:], ps_tr[:])
                # denominators: pair-sum the per-segment exp accumulators
                for t in range(NT):
                    dn = sp4.tile([P, 16], f32, tag="dn", name="dn")
                    nc.vector.tensor_reduce(
                        dn[:], acc2[t][:].rearrange("p (h s) -> p h s", s=2),
                        axis=AX.X, op=ALU.add,
                    )
                    nc.vector.tensor_scalar_add(dn[:], dn[:], qcount[:, t : t + 1])
                    nc.vector.reciprocal(dn[:], dn[:])
                    ps_t2 = ptr2.tile([16, P], f32, tag="tr2", name="tr2")
                    nc.tensor.transpose(ps_t2[:], dn[:], ident[:])
                    nc.vector.tensor_copy(rdenom_fm[0:16, P * t : P * (t + 1)], ps_t2[:])
            phn.close()

            es_k.close()
            es_q.close()

            # read back per-tile V sums (needed only for phase NUM)
            csum_all = cp.tile([32, T], bf16, tag="csum_all", name="csum_all")
            nc.sync.dma_start(csum_all[:], kv2_out[:])

            # ================= Phase NUM =================
            attn = [None] * DC
            es_h2n = ExitStack()
            h2np = es_h2n.enter_context(tc.tile_pool(name="h2npool", bufs=8, side="right"))
            es_h2 = ExitStack()
            h2p = es_h2.enter_context(tc.tile_pool(name="h2pool", bufs=8, side="right"))
            es_attn = ExitStack()
            ap = es_attn.enter_context(tc.tile_pool(name="attnpool", bufs=8, side="right"))
            ph = ExitStack()
            vdp = ph.enter_context(tc.tile_pool(name="vdpool", bufs=4))
            pn = ph.enter_context(tc.tile_pool(name="ps_num", bufs=5, space="PSUM"))
            prd = ph.enter_context(tc.tile_pool(name="ps_rd", bufs=2, space="PSUM"))
            pp = ph.enter_context(tc.tile_pool(name="ps_p", bufs=1, space="PSUM"))
            if True:
                P_sb = [None] * NT
                for i in range(NT):
                    P_sb[i] = vdp.tile([1, D], bf16, tag=f"P_sb{i}", name=f"P_sb{i}", bufs=1)
                    for half in range(2):
                        ps_P = pp.tile([1, T], f32, tag="pP", name="pP")
                        nc.tensor.matmul(
                            ps_P[:], w32[:, 2 * i + half : 2 * i + half + 1],
                            csum_all[:],
                            start=True, stop=True,
                        )
                        nc.vector.tensor_copy(
                            P_sb[i][0:1, T * half : T * (half + 1)], ps_P[:]
                        )
                Vd = [None] * NT
                for t in range(NT):
                    Vd[t] = vdp.tile([P, D], bf16, tag="Vd", name="Vd")
                    nc.vector.tensor_mul(
                        Vd[t][:].rearrange("p (h x) -> p h x", h=16),
                        V[t][:].rearrange("p (h x) -> p h x", h=16),
                        E16[t][:, :, None].broadcast_to([P, 16, HD]),
                    )
                for c in range(DC):
                    ps_n = pn.tile([P, T], f32, tag="n", name="n")
                    for i in range(NT):
                        sl = ps_n[:, P * i : P * (i + 1)]
                        nc.tensor.matmul(
                            sl, P_sb[i][0:1, P * c : P * (c + 1)],
                            onesrow[0:1, 0:P],
                            start=True, stop=False,
                        )
                        nc.tensor.matmul(
                            sl, V[i][:, P * c : P * (c + 1)], L128[:],
                            start=False, stop=False,
                        )
                        nc.tensor.matmul(
                            sl, Vd[i][:, P * c : P * (c + 1)], I128b[:],
                            start=False, stop=True,
                        )
                    ps_r = prd.tile([P, T], f32, tag="rd", name="rd")
                    nc.tensor.matmul(
                        ps_r[:], H16b[:, P * c : P * (c + 1)], rdenom_fm[:],
                        start=True, stop=True,
                    )
                    rd_sb = scr.tile([P, T], f32, tag="rds", name="rds")
                    nc.scalar.copy(rd_sb[:], ps_r[:])
                    attn[c] = ap.tile([P, T], bf16, tag="attn", name="attn")
                    nc.vector.tensor_mul(attn[c][:], ps_n[:], rd_sb[:])

            ph.close()
            es_v.close()

            # ================= Phase WO (+ residual) =================
            h2 = [None] * DC
            ph = ExitStack()
            pw = ph.enter_context(tc.tile_pool(name="ps_wo", bufs=8, space="PSUM"))
            if True:
                psum = [None] * DC
                for k in range(DC):
                    wt = wp.tile([P, D], bf16, tag="w", name="w")
                    nc.sync.dma_start(wt[:], wo_d[P * k : P * (k + 1), :])
                    for m in range(DC):
                        if k == 0:
                            psum[m] = pw.tile([P, T], f32, tag="wo", name="wo")
                        nc.tensor.matmul(
                            psum[m][:], wt[:, P * m : P * (m + 1)], attn[k][:],
                            start=(k == 0), stop=(k == DC - 1),
                        )
                for m in range(DC):
                    t_ = h2p.tile([P, T], bf16, tag="h2", name="h2")
                    nc.vector.scalar_tensor_tensor(
                        t_[:], psum[m][:], boc[:, m : m + 1],
                        ho[m][:], ALU.add, ALU.add,
                    )
                    h2[m] = t_

            ph.close()
            es_attn.close()

            # ================= Phase LN2 =================
            ph = ExitStack()
            pln2 = ph.enter_context(tc.tile_pool(name="ps_ln2", bufs=1, space="PSUM"))
            h2n = layer_norm_block(pln2, h2, h2np, "h2n")
            ph.close()
            es_h2.close()

            # ================= Phase FFN1 =================
            a1 = [None] * (DFF // P)
            ph = ExitStack()
            es_a1 = ExitStack()
            a1p = es_a1.enter_context(tc.tile_pool(name="a1pool", bufs=32))
            pf1 = ph.enter_context(tc.tile_pool(name="ps_f1", bufs=8, space="PSUM"))
            if True:
                for g in range(DFF // P // DC):
                    psum = [None] * DC
                    for k in range(DC):
                        wt = wp.tile([P, D], bf16, tag="w", name="w")
                        nc.sync.dma_start(
                            wt[:], w1_d[P * k : P * (k + 1), D * g : D * (g + 1)]
                        )
                        for m in range(DC):
                            if k == 0:
                                psum[m] = pf1.tile([P, T], f32, tag="f1", name="f1")
                            nc.tensor.matmul(
                                psum[m][:], wt[:, P * m : P * (m + 1)], h2n[k][:],
                                start=(k == 0), stop=(k == DC - 1),
                            )
                    for m in range(DC):
                        idx = DC * g + m
                        a1[idx] = a1p.tile([P, T], bf16, tag="a1", name="a1")
                        nc.vector.tensor_scalar(
                            a1[idx][:], psum[m][:], b1c[:, idx : idx + 1], 0.0,
                            ALU.add, ALU.max,
                        )

            ph.close()
            es_h2n.close()
            es_ho.close()

            # ================= Phase FFN2 =================
            ph = ExitStack()
            op = ph.enter_context(tc.tile_pool(name="opool", bufs=8))
            pf2 = ph.enter_context(tc.tile_pool(name="ps_f2", bufs=8, space="PSUM"))
            if True:
                psum = [None] * DC
                for k in range(DFF // P):
                    wt = wp.tile([P, D], bf16, tag="w", name="w")
                    nc.sync.dma_start(wt[:], w2_d[P * k : P * (k + 1), :])
                    for m in range(DC):
                        if k == 0:
                            psum[m] = pf2.tile([P, T], f32, tag="f2", name="f2")
                        nc.tensor.matmul(
                            psum[m][:], wt[:, P * m : P * (m + 1)], a1[k][:],
                            start=(k == 0), stop=(k == DFF // P - 1),
                        )
                for m in range(DC):
                    o_ = op.tile([P, T], f32, tag="o", name="o")
                    nc.vector.tensor_scalar(
                        o_[:], psum[m][:], b2c[:, m : m + 1], 0.0,
                        ALU.add, ALU.max,
                    )
                    nc.sync.dma_start(out_d[P * m : P * (m + 1), :], o_[:])
            ph.close()
            es_a1.close()

    return nc


def _host_inputs(x, g1, be1, wq, bq, wk, bk, wv, bv, wo, bo, g2, be2,
                 w1, b1, w2, b2):
    f = np.float32
    bf = ml_dtypes.bfloat16
    x = np.asarray(x, f)

    def cols(v, n):
        return np.ascontiguousarray(np.asarray(v, f).reshape(n, P).T)

    g1 = np.asarray(g1, f); be1 = np.asarray(be1, f)
    g2 = np.asarray(g2, f); be2 = np.asarray(be2, f)
    wq = np.asarray(wq, f); wk = np.asarray(wk, f); wv = np.asarray(wv, f)
    wo = np.asarray(wo, f); w1 = np.asarray(w1, f); w2 = np.asarray(w2, f)
    # gamma/beta folds: the kernel's LN emits plain z = (x - m) * rstd.
    # h1 = g1*z1 + be1 reaches Q/V through ho = g1*z1 (bias be1@w folded into
    # bq/bv), reaches K through wk' = g1*wk (bias be1@wk folded into bk), and
    # reaches the residual via ho + bo' with bo' = bo + be1. h2n = g2*z2 + be2
    # reaches FFN1 through w1' = g2*w1 and b1' = b1 + be2@w1.
    shared = {
        "wq": wq.astype(bf), "wk": (g1[:, None] * wk).astype(bf),
        "wv": wv.astype(bf), "wo": wo.astype(bf),
        "w1": (g2[:, None] * w1).astype(bf), "w2": w2.astype(bf),
        "g1c": cols(g1, DC),
        "bqc": cols(np.asarray(bq, f) + be1 @ wq, DC),
        "bkc": cols(np.asarray(bk, f) + be1 @ wk, DC),
        "boc": cols(np.asarray(bo, f) + be1, DC),
        "b1c": cols(np.asarray(b1, f) + be2 @ w1, DFF // P),
        "b2c": cols(b2, DC),
        "bvrow": (np.asarray(bv, f).reshape(1, D)
                  + (be1 @ wv).reshape(1, D)).astype(bf),
        "L128": np.triu(np.ones((P, P), f), 1).astype(bf),
        "I128b": np.eye(P, dtype=f).astype(bf),
        "I128r": np.eye(P, dtype=f),
        "ident": np.eye(P, dtype=f),
        "onesrow": np.ones((1, S), f).astype(bf),
        "onescol": np.ones((P, 1), f).astype(bf),
    }
    H16T = np.zeros((P, P), f)
    H16b = np.zeros((16, D), f)
    for c in range(DC):
        for i in range(2):
            h = 2 * c + i
            H16T[HD * i : HD * (i + 1), 16 * c + h] = 1.0
            H16b[h, P * c + HD * i : P * c + HD * (i + 1)] = 1.0
    shared["H16T"] = H16T.astype(bf)
    shared["H16b"] = H16b

    in_maps = []
    for core in range(NCORES):
        b, j = core // CH, core % CH
        m = dict(shared)
        # full batch, feature-major, each 512-block rotated so the core's own
        # 128-tile sits at position 0: block s order = tiles [4s + (j+r)%4]
        xb = x[b]                                     # [S, D]
        blocks = []
        for s_ in range(NB):
            tiles = [xb[P * (CH * s_ + (j + r) % CH) : P * (CH * s_ + (j + r) % CH + 1), :]
                     for r in range(CH)]
            blocks.append(np.concatenate(tiles, axis=0))
        xperm = np.concatenate(blocks, axis=0)        # [S, D] permuted
        m["xT"] = np.ascontiguousarray(xperm.T).astype(bf)
        # qcount: global row index of own tile i, row p
        qc = np.stack(
            [P * (j + CH * i) + np.arange(P, dtype=f) for i in range(NT)], axis=1
        )
        m["qcount"] = np.ascontiguousarray(qc)
        # boundary-block mask in rotated coordinates: position c holds tile
        # r(c) = (j + c//128) % 4; keep iff 128*r(c) + (c%128) >= 128j + p
        c_ = np.arange(T)[None, :]
        p_ = np.arange(P)[:, None]
        rposc = (j + c_ // P) % CH
        keep = (P * rposc + (c_ % P)) >= (P * j + p_)
        m["M1"] = np.where(keep, 0.0, NEG).astype(f)
        # prefix weights: P_i sums vtsum over global tiles g' < j + 4*i,
        # AG row layout: rank r rows [8r+2i'+h'] = (g'=r+4i', half h')
        w32 = np.zeros((32, 8), f)
        for i in range(NT):
            for h_ in range(2):
                for r in range(CH):
                    for i2 in range(NT):
                        if r + CH * i2 < j + CH * i:
                            w32[8 * r + 2 * i2 + h_, 2 * i + h_] = 1.0
        m["w32"] = w32.astype(bf)
        in_maps.append(m)
    return in_maps


_nc_cache = None


def kernel(**inputs):
    global _nc_cache
    if _nc_cache is None:
        _nc_cache = _build()
    nc = _nc_cache
    in_maps = _host_inputs(**inputs)
    res = run_bass_kernel_spmd(nc, in_maps, list(range(NCORES)))
    out = np.empty((B, S, D), np.float32)
    for core in range(NCORES):
        b, j = core // CH, core % CH
        oT = res.results[core]["outT"]
        for i in range(NT):
            g = j + CH * i
            out[b, P * g : P * (g + 1), :] = oT[:, P * i : P * (i + 1)].T
    return out


def make_timed_runner(**inputs):
    """Build the program once and return (run_fn, assemble_fn) where run_fn()
    executes on the 8 cores re-using the compiled NEFF (for timing loops)."""
    import jax
    from jax.sharding import Mesh, PartitionSpec
    from jax.experimental.shard_map import shard_map
    from concourse import bass2jax

    global _nc_cache
    if _nc_cache is None:
        _nc_cache = _build()
    nc = _nc_cache
    in_maps = _host_inputs(**inputs)

    bass2jax.install_neuronx_cc_hook()
    partition_name = nc.partition_id_tensor.name if nc.partition_id_tensor else None
    in_names, out_names, out_avals, zero_outs = [], [], [], []
    for alloc in nc.m.functions[0].allocations:
        if not isinstance(alloc, mybir.MemoryLocationSet):
            continue
        name = alloc.memorylocations[0].name
        if alloc.kind == "ExternalInput":
            if name != partition_name:
                in_names.append(name)
        elif alloc.kind == "ExternalOutput":
            out_names.append(name)
            shape = tuple(alloc.tensor_shape)
            dtype = mybir.dt.np(alloc.dtype)
            out_avals.append(jax.core.ShapedArray(shape, dtype))
            zero_outs.append(np.zeros(shape, dtype))
    n_params = len(in_names)
    all_in = in_names + out_names
    if partition_name is not None:
        all_in.append(partition_name)

    def _body(*args):
        operands = list(args)
        if partition_name is not None:
            operands.append(bass2jax.partition_id_tensor())
        outs = bass2jax._bass_exec_p.bind(
            *operands,
            out_avals=tuple(out_avals),
            in_names=tuple(all_in[: n_params + len(out_names) + (0 if partition_name is None else 1)]),
            out_names=tuple(out_names),
            lowering_input_output_aliases=(),
            sim_require_finite=True,
            sim_require_nnan=True,
            nc=nc,
        )
        return tuple(outs)

    devices = jax.devices()[:NCORES]
    mesh = Mesh(np.asarray(devices), ("core",))
    nin = n_params + len(out_names)
    sharded = jax.jit(
        shard_map(
            _body, mesh=mesh,
            in_specs=(PartitionSpec("core"),) * nin,
            out_specs=(PartitionSpec("core"),) * len(out_names),
            check_rep=False,
        ),
        keep_unused=True,
    )
    concat_in = [
        np.concatenate([np.asarray(in_maps[c][nm]) for c in range(NCORES)], axis=0)
        for nm in in_names
    ]
    concat_zeros = [
        np.zeros((NCORES * z.shape[0], *z.shape[1:]), z.dtype) for z in zero_outs
    ]
    from jax.sharding import NamedSharding
    sh = NamedSharding(mesh, PartitionSpec("core"))
    args = [jax.device_put(a, sh) for a in concat_in + concat_zeros]

    def run():
        outs = sharded(*args)
        jax.block_until_ready(outs)
        return outs

    def run_async():
        return sharded(*args)

    def assemble(outs):
        res = np.asarray(outs[out_names.index("outT")]).reshape(NCORES, D, T)
        out = np.empty((B, S, D), np.float32)
        for core in range(NCORES):
            b, j = core // CH, core % CH
            for i in range(NT):
                g = j + CH * i
                out[b, P * g : P * (g + 1), :] = res[core][:, P * i : P * (i + 1)].T
        return out

    run.run_async = run_async
    return run, assemble


# revision 27
# speedup vs baseline: 1.0842x; 1.0267x over previous
"""Trainium2 Bass kernel for nn_DecoderBlock (B=2, S=2048, D=1024, DFF=4096, H=16).

Sharding: 8 cores = 2 batches x 4 cores. Each core owns 512 interleaved
128-token q-tiles (core j of a batch owns global tiles j, j+4, j+8, j+12) but
receives the FULL batch sequence (feature-major, bf16) with each 512-token
block rotated so the core's own 128-token tile sits at position 0 of every
block. LayerNorm1 and the K projection are computed redundantly for the full
sequence on every core, which removes the K AllGather entirely (the cost
model charges 15us fixed latency per collective; four sliced 1MB gathers
serialized at ~41us each on the collective engine in the baseline). The only
remaining collective is the tiny per-tile V-column-sum AllGather (32KB).

The reference module's triu/transpose softmax degenerates mathematically:
    c[q,k] = 1/denom(q)            for k < q
           = exp(s[q,q])/denom(q)  for k == q
           = 0                     for k > q
    denom(q) = q + sum_{k>=q} exp(s[q,k])
so attention output = (prefix_sum(V) + exp(diag)*V_own) / denom, and V never
crosses cores - only per-128-row-tile V column sums.

Numerics: all big matmuls run in bf16 (weights shipped bf16 from host), PSUM
accumulation stays f32, LayerNorm statistics stay f32. The small [*,128]-free
matmuls of the numerator phase run bf16 (f32r below 256 free columns costs 4
cycles/row on TRN2). Score exp row-sums use the activation engine's fused
accumulator instead of DVE reductions.

LayerNorm gamma/beta are folded into the downstream weights on the host
(wk' = g1*wk, bk' = bk + be1@wk, etc.; the residual picks up g1 via the
own-token extraction and be1 via bo' = bo + be1), so the on-chip LN applies
only (x - mean) * rstd (two DVE/Pool passes). The V projection runs with
resident wv in a 2-PSUM-bank footprint and is emission-interleaved with the
score loop (together with V column sums and the diagonal pass) so the tensor
engine fills the activation-bound exp window.
"""
import sys

sys.path.insert(0, "/opt/trn_rl_repo")

import ml_dtypes
import numpy as np

from contextlib import ExitStack

import concourse.bass as bass
import concourse.mybir as mybir
import concourse.tile as tile
from concourse.bass_utils import run_bass_kernel_spmd
from concourse.vector_clock import ScopedClock

# ---------------------------------------------------------------------------
# Patch for this walrus build: it rejects more than one sync-wait command per
# instruction. Split multi-wait instructions into preceding same-engine NOPs
# (program order on the engine preserves semantics), both for scheduled
# instructions and for the TileContext tail drain.
# ---------------------------------------------------------------------------
_MAX_WAITS = 1
_orig_lower = tile.TileContext._lower_ordered_insts


def _split_waits(ordered):
    for bb_name, insts in ordered.items():
        new_insts = []
        for inst in insts:
            si = inst.sync_info
            if si is not None and si.on_wait and len(si.on_wait) > _MAX_WAITS:
                waits = list(si.on_wait)
                for i, w in enumerate(waits[:-_MAX_WAITS]):
                    new_insts.append(
                        mybir.InstNoOp(
                            name=f"{inst.name}-ws{i}",
                            sync_info=mybir.SyncInfo(on_wait=[w], on_update=[]),
                            bass_nofuse=True,
                            engine=inst.engine,
                        )
                    )
                inst.sync_info = mybir.SyncInfo(
                    on_wait=waits[-_MAX_WAITS:],
                    on_update=list(si.on_update) if si.on_update else [],
                )
            new_insts.append(inst)
        ordered[bb_name] = new_insts
    return ordered


def _lower_ordered_insts(self, ordered):
    return _orig_lower(self, _split_waits(ordered))


def _drain_and_barrier(self, tick_clock, wait_clock):
    nc = self.nc
    drain_inst = nc.sync.drain()
    wait_clock.add_sem_waits(
        drain_inst.ins, ScopedClock({None: tick_clock.global_clock})
    )
    si = drain_inst.ins.sync_info
    waits = list(si.on_wait) if si is not None else []
    if len(waits) > _MAX_WAITS:
        drain_inst.ins.sync_info = mybir.SyncInfo(
            on_wait=waits[:_MAX_WAITS],
            on_update=list(si.on_update) if si.on_update else [],
        )
        for i in range(_MAX_WAITS, len(waits), _MAX_WAITS):
            nop = nc.sync.nop(nofuse=True)
            nop.ins.sync_info = mybir.SyncInfo(
                on_wait=waits[i : i + _MAX_WAITS], on_update=[]
            )
    nc.all_engine_barrier()
    assert self.sems is not None
    popped = nc._tile_sem_poison_stack.pop()
    assert popped is self._sem_poison
    nc.clear_and_free_semaphores(list(self.sems.allocated().values()))
    nc.all_engine_barrier()


tile.TileContext._lower_ordered_insts = _lower_ordered_insts
tile.TileContext._drain_and_barrier = _drain_and_barrier

# ---------------------------------------------------------------------------

B, S, D, DFF, H = 2, 2048, 1024, 4096, 16
HD = D // H          # 64
EPS = 1e-5
NCORES = 8
CH = 4               # cores per batch / 128-tiles per 512-block
T = S // CH          # 512 own tokens per core
P = 128
NT = T // P          # 4 own q-tiles per core
NB = S // T          # 4 global 512-token blocks
DC = D // P          # 8 feature chunks

f32 = mybir.dt.float32
f32r = mybir.dt.float32r
bf16 = mybir.dt.bfloat16
AF = mybir.ActivationFunctionType
ALU = mybir.AluOpType
AX = mybir.AxisListType
NEG = -1.0e9


def _build():
    nc = bass.Bass(num_devices=NCORES)

    def par(name, shape, dt):
        return nc.declare_dram_parameter(name, shape, dt, isOutput=False)

    # per-core data
    xT_d = par("xT", [D, S], bf16)                # full batch, block-rotated
    qcount_d = par("qcount", [P, NT], f32)        # col i = global row index of tile i
    M1_d = par("M1", [P, T], f32r)                # boundary-block additive mask
    w32_d = par("w32", [32, 8], bf16)             # prefix tile-sum weights
    # shared weights (natural [din, dout] layout = lhsT), bf16
    wq_d = par("wq", [D, D], bf16)
    wk_d = par("wk", [D, D], bf16)
    wv_d = par("wv", [D, D], bf16)
    wo_d = par("wo", [D, D], bf16)
    w1_d = par("w1", [D, DFF], bf16)
    w2_d = par("w2", [DFF, D], bf16)
    # per-partition gamma/bias columns ([P, n_chunks], f32)
    g1_d = par("g1c", [P, DC], f32)
    bq_d = par("bqc", [P, DC], f32)
    bk_d = par("bkc", [P, DC], f32)
    bo_d = par("boc", [P, DC], f32)
    b1_d = par("b1c", [P, DFF // P], f32)
    b2_d = par("b2c", [P, DC], f32)
    bv_d = par("bvrow", [1, D], bf16)             # V bias as a row (free-dim)
    # shared constant matrices
    L128_d = par("L128", [P, P], bf16)            # L[k,q] = 1 if k < q
    I128b_d = par("I128b", [P, P], bf16)          # identity (bf16, NUM phase)
    I128r_d = par("I128r", [P, P], f32r)          # identity (f32r, mask matmul)
    ident_d = par("ident", [P, P], f32)
    H16T_d = par("H16T", [P, P], bf16)            # [:,16c+h]: head-h rows in chunk c
    H16b_d = par("H16b", [16, D], f32r)           # [h,128c+p]: head(p of chunk c)==h
    onesrow_d = par("onesrow", [1, S], bf16)
    onescol_d = par("onescol", [P, 1], bf16)

    out_d = nc.declare_dram_parameter("outT", [D, T], f32, isOutput=True)

    kv2_in = nc.dram_tensor("kv2_in", [8, T], bf16)
    kv2_out = nc.dram_tensor("kv2_out", [8 * CH, T], bf16)

    with tile.TileContext(nc, pool_alloc_mode="queue") as tc, ExitStack() as es:
            cp = es.enter_context(tc.tile_pool(name="cpool", bufs=1))
            lnp = es.enter_context(tc.tile_pool(name="lnstat", bufs=1))
            sp4 = es.enter_context(tc.tile_pool(name="small4", bufs=4))
            scr = es.enter_context(tc.tile_pool(name="scr", bufs=2))
            wp = es.enter_context(tc.tile_pool(name="wstream", bufs=8))
            es_ho = ExitStack()
            hop = es_ho.enter_context(tc.tile_pool(name="hopool", bufs=8, side="right"))

            # ---- load constants ----
            def load(pool, name, src, shape, dt, tag=None):
                t_ = pool.tile(shape, dt, tag=tag or name, name=tag or name)
                nc.sync.dma_start(t_[:], src[:])
                return t_

            qcount = load(cp, "qcount", qcount_d, [P, NT], f32)
            M1 = load(cp, "M1", M1_d, [P, T], f32r)
            w32 = load(cp, "w32", w32_d, [32, 8], bf16)
            g1c = load(cp, "g1c", g1_d, [P, DC], f32)
            bqc = load(cp, "bqc", bq_d, [P, DC], f32)
            bkc = load(cp, "bkc", bk_d, [P, DC], f32)
            boc = load(cp, "boc", bo_d, [P, DC], f32)
            b1c = load(cp, "b1c", b1_d, [P, DFF // P], f32)
            b2c = load(cp, "b2c", b2_d, [P, DC], f32)
            bvrow = load(cp, "bvrow", bv_d, [1, D], bf16)
            ident = load(cp, "ident", ident_d, [P, P], f32)
            H16T = load(cp, "H16T", H16T_d, [P, P], bf16)
            H16b = load(cp, "H16b", H16b_d, [16, D], f32r)
            onesrow = load(cp, "onesrow", onesrow_d, [1, S], bf16)
            onescol = load(cp, "onescol", onescol_d, [P, 1], bf16)
            L128 = load(cp, "L128", L128_d, [P, P], bf16)
            I128b = load(cp, "I128b", I128b_d, [P, P], bf16)
            I128r = load(cp, "I128r", I128r_d, [P, P], f32r)
            epsc = cp.tile([1, 1], f32, tag="epsc", name="epsc")
            nc.vector.memset(epsc[:], EPS)

            # ---- LayerNorm for ONE 512-token block (feature-major) ----
            # Produces plain (x - mean) * rstd; gamma/beta are folded into the
            # downstream weights on the host.
            def layer_norm_block(ps_pool, xblk, out_pool, out_tag):
                sq = []
                for k in range(DC):
                    s_ = scr.tile([P, T], bf16, tag="lnsq", name="lnsq", bufs=8)
                    nc.scalar.activation(s_[:], xblk[k][:], AF.Square)
                    sq.append(s_)
                ps_sum = ps_pool.tile([1, T], f32, tag="ln_sum", name="ln_sum")
                for k in range(DC):
                    nc.tensor.matmul(ps_sum[:], onescol[:], xblk[k][:],
                                     start=(k == 0), stop=(k == DC - 1))
                ps_sq = ps_pool.tile([1, T], f32, tag="ln_sq", name="ln_sq")
                for k in range(DC):
                    nc.tensor.matmul(ps_sq[:], onescol[:], sq[k][:],
                                     start=(k == 0), stop=(k == DC - 1))
                mean = lnp.tile([1, T], f32, tag="ln_mean", name="ln_mean")
                nc.vector.tensor_scalar_mul(mean[:], ps_sum[:], 1.0 / D)
                msq = lnp.tile([1, T], f32, tag="ln_msq", name="ln_msq")
                nc.vector.tensor_scalar_mul(msq[:], ps_sq[:], 1.0 / D)
                m2 = lnp.tile([1, T], f32, tag="ln_m2", name="ln_m2")
                nc.vector.tensor_mul(m2[:], mean[:], mean[:])
                var = lnp.tile([1, T], f32, tag="ln_var", name="ln_var")
                nc.vector.tensor_sub(var[:], msq[:], m2[:])
                sd = lnp.tile([1, T], f32, tag="ln_sd", name="ln_sd")
                nc.scalar.activation(sd[:], var[:], AF.Sqrt, bias=epsc[:])
                rstd = lnp.tile([1, T], f32, tag="ln_rstd", name="ln_rstd")
                nc.vector.reciprocal(rstd[:], sd[:])
                mrs = lnp.tile([1, T], f32, tag="ln_mrs", name="ln_mrs")
                nc.vector.tensor_mul(mrs[:], mean[:], rstd[:])
                rstd_b = lnp.tile([1, T], bf16, tag="ln_rstdb", name="ln_rstdb")
                nc.vector.tensor_copy(rstd_b[:], rstd[:])
                mrs_b = lnp.tile([1, T], bf16, tag="ln_mrsb", name="ln_mrsb")
                nc.vector.tensor_copy(mrs_b[:], mrs[:])
                ps_R = ps_pool.tile([P, T], f32, tag="ln_Rb", name="ln_Rb")
                nc.tensor.matmul(ps_R[:], onesrow[0:1, 0:P], rstd_b[:],
                                 start=True, stop=True)
                ps_M = ps_pool.tile([P, T], f32, tag="ln_Mb", name="ln_Mb")
                nc.tensor.matmul(ps_M[:], onesrow[0:1, 0:P], mrs_b[:],
                                 start=True, stop=True)
                R_b = scr.tile([P, T], bf16, tag="ln_Rsb", name="ln_Rsb")
                nc.scalar.copy(R_b[:], ps_R[:])
                M_b = scr.tile([P, T], bf16, tag="ln_Msb", name="ln_Msb")
                nc.scalar.copy(M_b[:], ps_M[:])
                outs = []
                for k in range(DC):
                    t1 = scr.tile([P, T], bf16, tag="lnt", name="lnt", bufs=2)
                    nc.vector.tensor_mul(t1[:], xblk[k][:], R_b[:])
                    o_ = out_pool.tile([P, T], bf16, tag=out_tag, name=out_tag)
                    nc.vector.tensor_sub(o_[:], t1[:], M_b[:])
                    outs.append(o_)
                return outs

            # ========= Phase LN1 + K, fused per 512-token block =========
            # LN of block b+1 (DVE/Act) overlaps the K projection of block b
            # (PE). PSUM: LN stats/broadcast 4 banks (bufs=1) + K 4 banks.
            es_hT = ExitStack()
            hp = es_hT.enter_context(tc.tile_pool(name="hpool", bufs=32))
            es_k = ExitStack()
            kfp = es_k.enter_context(tc.tile_pool(name="kfpool", bufs=32, side="right"))
            ph = ExitStack()
            xp = ph.enter_context(tc.tile_pool(name="xpool", bufs=12))
            wkp = ph.enter_context(tc.tile_pool(name="wkres", bufs=8))
            pln = ph.enter_context(tc.tile_pool(name="ps_ln1", bufs=1, space="PSUM"))
            pk = ph.enter_context(tc.tile_pool(name="ps_k", bufs=4, space="PSUM"))
            wkt = []
            for k in range(DC):
                wt = wkp.tile([P, D], bf16, tag="wk", name="wk")
                nc.sync.dma_start(wt[:], wk_d[P * k : P * (k + 1), :])
                wkt.append(wt)
            xin = [[None] * NB for _ in range(DC)]
            for blk in range(NB):
                for k in range(DC):
                    t_ = xp.tile([P, T], bf16, tag="xT", name="xT")
                    nc.sync.dma_start(
                        t_[:], xT_d[P * k : P * (k + 1), T * blk : T * (blk + 1)]
                    )
                    xin[k][blk] = t_
            hT = [[None] * NB for _ in range(DC)]
            K_sb = [[None] * NB for _ in range(DC)]
            ho = [None] * DC
            for k in range(DC):
                ho[k] = hop.tile([P, T], bf16, tag="ho", name="ho")
            for blk in range(NB):
                houts = layer_norm_block(pln, [xin[k][blk] for k in range(DC)],
                                         hp, "hT")
                for k in range(DC):
                    hT[k][blk] = houts[k]
                    # own-token extraction with the g1 fold (own tile = block
                    # position 0): ho[k][:, 128*blk:] = g1_k * hT[k][blk][:, 0:128]
                    nc.vector.tensor_scalar_mul(
                        ho[k][:, P * blk : P * (blk + 1)], houts[k][:, 0:P],
                        g1c[:, k : k + 1],
                    )
                for half in range(2):
                    psum = [None] * 4
                    for k in range(DC):
                        for mi in range(4):
                            m = 4 * half + mi
                            if k == 0:
                                psum[mi] = pk.tile([P, T], f32, tag="kp", name="kp")
                            nc.tensor.matmul(
                                psum[mi][:], wkt[k][:, P * m : P * (m + 1)],
                                hT[k][blk][:],
                                start=(k == 0), stop=(k == DC - 1),
                            )
                    for mi in range(4):
                        m = 4 * half + mi
                        o_ = kfp.tile([P, T], bf16, tag="K", name="K")
                        nc.vector.tensor_scalar_add(o_[:], psum[mi][:], bkc[:, m : m + 1])
                        K_sb[m][blk] = o_
            ph.close()
            es_hT.close()

            # ================= Phase Q (own tokens, feature-major) =================
            es_v = ExitStack()
            vp = es_v.enter_context(tc.tile_pool(name="vpool", bufs=4))
            wvp = es_v.enter_context(tc.tile_pool(name="wvres", bufs=8))
            es_q = ExitStack()
            qp = es_q.enter_context(tc.tile_pool(name="qpool", bufs=8))
            ph = ExitStack()
            pq = ph.enter_context(tc.tile_pool(name="ps_qv", bufs=8, space="PSUM"))
            if True:
                psum = [None] * DC
                for k in range(DC):
                    wt = wp.tile([P, D], bf16, tag="w", name="w")
                    nc.sync.dma_start(wt[:], wq_d[P * k : P * (k + 1), :])
                    for m in range(DC):
                        if k == 0:
                            psum[m] = pq.tile([P, T], f32, tag="qv", name="qv")
                        nc.tensor.matmul(
                            psum[m][:], wt[:, P * m : P * (m + 1)], ho[k][:],
                            start=(k == 0), stop=(k == DC - 1),
                        )
                Q = [None] * DC
                for m in range(DC):
                    Q[m] = qp.tile([P, T], bf16, tag="Q", name="Q")
                    nc.vector.tensor_scalar_add(Q[m][:], psum[m][:], bqc[:, m : m + 1])
                # resident wv for the interleaved V projection
                wvt = []
                for k in range(DC):
                    wt = wvp.tile([P, D], bf16, tag="wv", name="wv")
                    nc.sync.dma_start(wt[:], wv_d[P * k : P * (k + 1), :])
                    wvt.append(wt)
            ph.close()

            # ============ Phase ATT: scores interleaved with V ============
            # Scores are exp-bound on the activation engine. Each (head,
            # q-tile) suffix is split into <=1024-col psum units so three can
            # be in flight (6 banks) while the V projection (2 banks) fills
            # the tensor engine between heads.
            acc2 = [sp4.tile([P, 2 * H], f32, tag="acc2", name="acc2")
                    for _ in range(NT)]
            for t in range(NT):
                nc.vector.memset(acc2[t][:], 0.0)
            E16 = [None] * NT
            V = [None] * NT
            rdenom_fm = cp.tile([16, T], f32r, tag="rdenom_fm", name="rdenom_fm")
            # (start, end) column ranges of the psum-local suffix per q-tile
            SEGS = [[(0, 1024), (1024, 2048)], [(0, 1024), (1024, 1536)],
                    [(0, 1024)], [(0, 512)]]
            phs = ExitStack()
            pa3 = phs.enter_context(tc.tile_pool(name="ps_att3", bufs=2, space="PSUM"))
            pvi = phs.enter_context(tc.tile_pool(name="ps_vi", bufs=2, space="PSUM"))
            pcs = phs.enter_context(tc.tile_pool(name="ps_cs", bufs=1, space="PSUM"))
            pa = phs.enter_context(tc.tile_pool(name="ps_att", bufs=1, space="PSUM"))
            if True:
                for t in range(NT):
                    V[t] = vp.tile([P, D], bf16, tag="V", name="V")
                ps_e = pa.tile([16, T], f32, tag="pe", name="pe")

                def emit_v_unit(u):          # u in 0..7: (t, n) V projection
                    t, n = u // 2, u % 2
                    ps = pvi.tile([P, T], f32, tag="vi", name="vi")
                    nc.tensor.matmul(
                        ps[:], onesrow[0:1, 0:P], bvrow[0:1, T * n : T * (n + 1)],
                        start=True, stop=False,
                    )
                    for k in range(DC):
                        nc.tensor.matmul(
                            ps[:], ho[k][:, P * t : P * (t + 1)],
                            wvt[k][:, T * n : T * (n + 1)],
                            start=False, stop=(k == DC - 1),
                        )
                    nc.vector.tensor_copy(V[t][:, T * n : T * (n + 1)], ps[:])

                def emit_cs_unit(i):         # V column sums for own tile i
                    for half in range(2):
                        ps_c = pcs.tile([1, T], f32, tag="cs", name="cs")
                        nc.tensor.matmul(
                            ps_c[:], onescol[:],
                            V[i][:, T * half : T * (half + 1)],
                            start=True, stop=True,
                        )
                        cs_scr = scr.tile([1, T], bf16, tag="cs_scr", name="cs_scr")
                        nc.vector.tensor_copy(cs_scr[:], ps_c[:])
                        nc.sync.dma_start(
                            kv2_in[2 * i + half : 2 * i + half + 1, :], cs_scr[:]
                        )

                def emit_e16_unit(c):        # diagonal pass, chunk c
                    Tt = scr.tile([P, T], bf16, tag="T", name="T")
                    for i in range(NT):
                        nc.vector.tensor_mul(
                            Tt[:, P * i : P * (i + 1)],
                            Q[c][:, P * i : P * (i + 1)],
                            K_sb[c][i][:, 0:P],
                        )
                    nc.tensor.matmul(
                        ps_e[:], H16T[:, 16 * c : 16 * (c + 1)], Tt[:],
                        start=(c == 0), stop=(c == DC - 1),
                    )

                for h in range(H):
                    c2, r0 = h // 2, HD * (h % 2)
                    for i in range(NT):
                        for sidx, (c0, c1) in enumerate(SEGS[i]):
                            ps_s = pa3.tile([P, 1024], f32, tag="s", name="s")
                            for kc in range(c0 // T, (c1 + T - 1) // T):
                                nc.tensor.matmul(
                                    ps_s[:, T * kc - c0 : T * (kc + 1) - c0],
                                    Q[c2][r0 : r0 + HD, P * i : P * (i + 1)],
                                    K_sb[c2][i + kc][r0 : r0 + HD, :],
                                    start=True, stop=(kc > 0),
                                )
                                if kc == 0:
                                    # boundary block: accumulate the causal
                                    # mask (exp's 1/32 scale -> NEG/32 ~ -3e7)
                                    nc.tensor.matmul(
                                        ps_s[:, 0:T], I128r[:], M1[:],
                                        start=False, stop=True,
                                    )
                            esc = scr.tile([P, 1024], bf16, tag="esc", name="esc",
                                           bufs=3)
                            nc.scalar.activation(
                                esc[:, 0 : c1 - c0], ps_s[:, 0 : c1 - c0],
                                AF.Exp, scale=1.0 / 32.0,
                                accum_out=acc2[i][:, 2 * h + sidx : 2 * h + sidx + 1],
                            )
                    # fill work between heads
                    if h < 8:
                        emit_v_unit(h)
                    elif h < 12:
                        emit_cs_unit(h - 8)
                        if h == 11:
                            nc.gpsimd.collective_compute(
                                "AllGather", ALU.bypass,
                                replica_groups=[[0, 1, 2, 3], [4, 5, 6, 7]],
                                ins=[kv2_in[:]], outs=[kv2_out[:]],
                            )
                    else:
                        emit_e16_unit(2 * (h - 12))
                        emit_e16_unit(2 * (h - 12) + 1)
                e16_sb = cp.tile([16, T], f32, tag="e16_sb", name="e16_sb")
                nc.scalar.activation(e16_sb[:], ps_e[:], AF.Exp, scale=1.0 / 32.0)
            phs.close()

            # ---- E16 transposes + denominators ----
            phn = ExitStack()
            ptr2 = phn.enter_context(tc.tile_pool(name="ps_tr2", bufs=2, space="PSUM"))
            if True:
                for t in range(NT):
                    ps_tr = ptr2.tile([P, 16], f32, tag="tr1", name="tr1")
                    nc.tensor.transpose(
                        ps_tr[:], e16_sb[0:16, P * t : P * (t + 1)],
                        ident[0:16, 0:16],
                    )
                    E16[t] = sp4.tile([P, 16], bf16, tag="E16", name="E16")
                    nc.vector.tensor_copy(E16[t][:], ps_tr[:])
                # denominators: pair-sum the per-segment exp accumulators
                for t in range(NT):
                    dn = sp4.tile([P, 16], f32, tag="dn", name="dn")
                    nc.vector.tensor_reduce(
                        dn[:], acc2[t][:].rearrange("p (h s) -> p h s", s=2),
                        axis=AX.X, op=ALU.add,
                    )
                    nc.vector.tensor_scalar_add(dn[:], dn[:], qcount[:, t : t + 1])
                    nc.vector.reciprocal(dn[:], dn[:])
                    ps_t2 = ptr2.tile([16, P], f32, tag="tr2", name="tr2")
                    nc.tensor.transpose(ps_t2[:], dn[:], ident[:])
                    nc.vector.tensor_copy(rdenom_fm[0:16, P * t : P * (t + 1)], ps_t2[:])
            phn.close()

            es_k.close()
            es_q.close()

            # read back per-tile V sums (needed only for phase NUM)
            csum_all = cp.tile([32, T], bf16, tag="csum_all", name="csum_all")
            nc.sync.dma_start(csum_all[:], kv2_out[:])

            # ================= Phase NUM =================
            attn = [None] * DC
            es_h2n = ExitStack()
            h2np = es_h2n.enter_context(tc.tile_pool(name="h2npool", bufs=8, side="right"))
            es_h2 = ExitStack()
            h2p = es_h2.enter_context(tc.tile_pool(name="h2pool", bufs=8, side="right"))
            es_attn = ExitStack()
            ap = es_attn.enter_context(tc.tile_pool(name="attnpool", bufs=8, side="right"))
            ph = ExitStack()
            vdp = ph.enter_context(tc.tile_pool(name="vdpool", bufs=4))
            pn = ph.enter_context(tc.tile_pool(name="ps_num", bufs=5, space="PSUM"))
            prd = ph.enter_context(tc.tile_pool(name="ps_rd", bufs=2, space="PSUM"))
            pp = ph.enter_context(tc.tile_pool(name="ps_p", bufs=1, space="PSUM"))
            if True:
                P_sb = [None] * NT
                for i in range(NT):
                    P_sb[i] = vdp.tile([1, D], bf16, tag=f"P_sb{i}", name=f"P_sb{i}", bufs=1)
                    for half in range(2):
                        ps_P = pp.tile([1, T], f32, tag="pP", name="pP")
                        nc.tensor.matmul(
                            ps_P[:], w32[:, 2 * i + half : 2 * i + half + 1],
                            csum_all[:],
                            start=True, stop=True,
                        )
                        nc.vector.tensor_copy(
                            P_sb[i][0:1, T * half : T * (half + 1)], ps_P[:]
                        )
                Vd = [None] * NT
                for t in range(NT):
                    Vd[t] = vdp.tile([P, D], bf16, tag="Vd", name="Vd")
                    nc.vector.tensor_mul(
                        Vd[t][:].rearrange("p (h x) -> p h x", h=16),
                        V[t][:].rearrange("p (h x) -> p h x", h=16),
                        E16[t][:, :, None].broadcast_to([P, 16, HD]),
                    )
                for c in range(DC):
                    ps_n = pn.tile([P, T], f32, tag="n", name="n")
                    for i in range(NT):
                        sl = ps_n[:, P * i : P * (i + 1)]
                        nc.tensor.matmul(
                            sl, P_sb[i][0:1, P * c : P * (c + 1)],
                            onesrow[0:1, 0:P],
                            start=True, stop=False,
                        )
                        nc.tensor.matmul(
                            sl, V[i][:, P * c : P * (c + 1)], L128[:],
                            start=False, stop=False,
                        )
                        nc.tensor.matmul(
                            sl, Vd[i][:, P * c : P * (c + 1)], I128b[:],
                            start=False, stop=True,
                        )
                    ps_r = prd.tile([P, T], f32, tag="rd", name="rd")
                    nc.tensor.matmul(
                        ps_r[:], H16b[:, P * c : P * (c + 1)], rdenom_fm[:],
                        start=True, stop=True,
                    )
                    rd_sb = scr.tile([P, T], f32, tag="rds", name="rds")
                    nc.scalar.copy(rd_sb[:], ps_r[:])
                    attn[c] = ap.tile([P, T], bf16, tag="attn", name="attn")
                    nc.vector.tensor_mul(attn[c][:], ps_n[:], rd_sb[:])

            ph.close()
            es_v.close()

            # ================= Phase WO (+ residual) =================
            h2 = [None] * DC
            ph = ExitStack()
            pw = ph.enter_context(tc.tile_pool(name="ps_wo", bufs=8, space="PSUM"))
            if True:
                psum = [None] * DC
                for k in range(DC):
                    wt = wp.tile([P, D], bf16, tag="w", name="w")
                    nc.sync.dma_start(wt[:], wo_d[P * k : P * (k + 1), :])
                    for m in range(DC):
                        if k == 0:
                            psum[m] = pw.tile([P, T], f32, tag="wo", name="wo")
                        nc.tensor.matmul(
                            psum[m][:], wt[:, P * m : P * (m + 1)], attn[k][:],
                            start=(k == 0), stop=(k == DC - 1),
                        )
                for m in range(DC):
                    t_ = h2p.tile([P, T], bf16, tag="h2", name="h2")
                    nc.vector.scalar_tensor_tensor(
                        t_[:], psum[m][:], boc[:, m : m + 1],
                        ho[m][:], ALU.add, ALU.add,
                    )
                    h2[m] = t_

            ph.close()
            es_attn.close()

            # ================= Phase LN2 =================
            ph = ExitStack()
            pln2 = ph.enter_context(tc.tile_pool(name="ps_ln2", bufs=1, space="PSUM"))
            h2n = layer_norm_block(pln2, h2, h2np, "h2n")
            ph.close()
            es_h2.close()

            # ================= Phase FFN1 =================
            a1 = [None] * (DFF // P)
            ph = ExitStack()
            es_a1 = ExitStack()
            a1p = es_a1.enter_context(tc.tile_pool(name="a1pool", bufs=32))
            pf1 = ph.enter_context(tc.tile_pool(name="ps_f1", bufs=8, space="PSUM"))
            if True:
                for g in range(DFF // P // DC):
                    psum = [None] * DC
                    for k in range(DC):
                        wt = wp.tile([P, D], bf16, tag="w", name="w")
                        nc.sync.dma_start(
                            wt[:], w1_d[P * k : P * (k + 1), D * g : D * (g + 1)]
                        )
                        for m in range(DC):
                            if k == 0:
                                psum[m] = pf1.tile([P, T], f32, tag="f1", name="f1")
                            nc.tensor.matmul(
                                psum[m][:], wt[:, P * m : P * (m + 1)], h2n[k][:],
                                start=(k == 0), stop=(k == DC - 1),
                            )
                    for m in range(DC):
                        idx = DC * g + m
                        a1[idx] = a1p.tile([P, T], bf16, tag="a1", name="a1")
                        nc.vector.tensor_scalar(
                            a1[idx][:], psum[m][:], b1c[:, idx : idx + 1], 0.0,
                            ALU.add, ALU.max,
                        )

            ph.close()
            es_h2n.close()
            es_ho.close()

            # ================= Phase FFN2 =================
            ph = ExitStack()
            op = ph.enter_context(tc.tile_pool(name="opool", bufs=8))
            pf2 = ph.enter_context(tc.tile_pool(name="ps_f2", bufs=8, space="PSUM"))
            if True:
                psum = [None] * DC
                for k in range(DFF // P):
                    wt = wp.tile([P, D], bf16, tag="w", name="w")
                    nc.sync.dma_start(wt[:], w2_d[P * k : P * (k + 1), :])
                    for m in range(DC):
                        if k == 0:
                            psum[m] = pf2.tile([P, T], f32, tag="f2", name="f2")
                        nc.tensor.matmul(
                            psum[m][:], wt[:, P * m : P * (m + 1)], a1[k][:],
                            start=(k == 0), stop=(k == DFF // P - 1),
                        )
                for m in range(DC):
                    o_ = op.tile([P, T], f32, tag="o", name="o")
                    nc.vector.tensor_scalar(
                        o_[:], psum[m][:], b2c[:, m : m + 1], 0.0,
                        ALU.add, ALU.max,
                    )
                    nc.sync.dma_start(out_d[P * m : P * (m + 1), :], o_[:])
            ph.close()
            es_a1.close()

    return nc


def _host_inputs(x, g1, be1, wq, bq, wk, bk, wv, bv, wo, bo, g2, be2,
                 w1, b1, w2, b2):
    f = np.float32
    bf = ml_dtypes.bfloat16
    x = np.asarray(x, f)

    def cols(v, n):
        return np.ascontiguousarray(np.asarray(v, f).reshape(n, P).T)

    g1 = np.asarray(g1, f); be1 = np.asarray(be1, f)
    g2 = np.asarray(g2, f); be2 = np.asarray(be2, f)
    wq = np.asarray(wq, f); wk = np.asarray(wk, f); wv = np.asarray(wv, f)
    wo = np.asarray(wo, f); w1 = np.asarray(w1, f); w2 = np.asarray(w2, f)
    # gamma/beta folds: the kernel's LN emits plain z = (x - m) * rstd.
    # h1 = g1*z1 + be1 reaches Q/V through ho = g1*z1 (bias be1@w folded into
    # bq/bv), reaches K through wk' = g1*wk (bias be1@wk folded into bk), and
    # reaches the residual via ho + bo' with bo' = bo + be1. h2n = g2*z2 + be2
    # reaches FFN1 through w1' = g2*w1 and b1' = b1 + be2@w1.
    shared = {
        "wq": wq.astype(bf), "wk": (g1[:, None] * wk).astype(bf),
        "wv": wv.astype(bf), "wo": wo.astype(bf),
        "w1": (g2[:, None] * w1).astype(bf), "w2": w2.astype(bf),
        "g1c": cols(g1, DC),
        "bqc": cols(np.asarray(bq, f) + be1 @ wq, DC),
        "bkc": cols(np.asarray(bk, f) + be1 @ wk, DC),
        "boc": cols(np.asarray(bo, f) + be1, DC),
        "b1c": cols(np.asarray(b1, f) + be2 @ w1, DFF // P),
        "b2c": cols(b2, DC),
        "bvrow": (np.asarray(bv, f).reshape(1, D)
                  + (be1 @ wv).reshape(1, D)).astype(bf),
        "L128": np.triu(np.ones((P, P), f), 1).astype(bf),
        "I128b": np.eye(P, dtype=f).astype(bf),
        "I128r": np.eye(P, dtype=f),
        "ident": np.eye(P, dtype=f),
        "onesrow": np.ones((1, S), f).astype(bf),
        "onescol": np.ones((P, 1), f).astype(bf),
    }
    H16T = np.zeros((P, P), f)
    H16b = np.zeros((16, D), f)
    for c in range(DC):
        for i in range(2):
            h = 2 * c + i
            H16T[HD * i : HD * (i + 1), 16 * c + h] = 1.0
            H16b[h, P * c + HD * i : P * c + HD * (i + 1)] = 1.0
    shared["H16T"] = H16T.astype(bf)
    shared["H16b"] = H16b

    in_maps = []
    for core in range(NCORES):
        b, j = core // CH, core % CH
        m = dict(shared)
        # full batch, feature-major, each 512-block rotated so the core's own
        # 128-tile sits at position 0: block s order = tiles [4s + (j+r)%4]
        xb = x[b]                                     # [S, D]
        blocks = []
        for s_ in range(NB):
            tiles = [xb[P * (CH * s_ + (j + r) % CH) : P * (CH * s_ + (j + r) % CH + 1), :]
                     for r in range(CH)]
            blocks.append(np.concatenate(tiles, axis=0))
        xperm = np.concatenate(blocks, axis=0)        # [S, D] permuted
        m["xT"] = np.ascontiguousarray(xperm.T).astype(bf)
        # qcount: global row index of own tile i, row p
        qc = np.stack(
            [P * (j + CH * i) + np.arange(P, dtype=f) for i in range(NT)], axis=1
        )
        m["qcount"] = np.ascontiguousarray(qc)
        # boundary-block mask in rotated coordinates: position c holds tile
        # r(c) = (j + c//128) % 4; keep iff 128*r(c) + (c%128) >= 128j + p
        c_ = np.arange(T)[None, :]
        p_ = np.arange(P)[:, None]
        rposc = (j + c_ // P) % CH
        keep = (P * rposc + (c_ % P)) >= (P * j + p_)
        m["M1"] = np.where(keep, 0.0, NEG).astype(f)
        # prefix weights: P_i sums vtsum over global tiles g' < j + 4*i,
        # AG row layout: rank r rows [8r+2i'+h'] = (g'=r+4i', half h')
        w32 = np.zeros((32, 8), f)
        for i in range(NT):
            for h_ in range(2):
                for r in range(CH):
                    for i2 in range(NT):
                        if r + CH * i2 < j + CH * i:
                            w32[8 * r + 2 * i2 + h_, 2 * i + h_] = 1.0
        m["w32"] = w32.astype(bf)
        in_maps.append(m)
    return in_maps


_nc_cache = None


def kernel(**inputs):
    global _nc_cache
    if _nc_cache is None:
        _nc_cache = _build()
    nc = _nc_cache
    in_maps = _host_inputs(**inputs)
    res = run_bass_kernel_spmd(nc, in_maps, list(range(NCORES)))
    out = np.empty((B, S, D), np.float32)
    for core in range(NCORES):
        b, j = core // CH, core % CH
        oT = res.results[core]["outT"]
        for i in range(NT):
            g = j + CH * i
            out[b, P * g : P * (g + 1), :] = oT[:, P * i : P * (i + 1)].T
    return out


def make_timed_runner(**inputs):
    """Build the program once and return (run_fn, assemble_fn) where run_fn()
    executes on the 8 cores re-using the compiled NEFF (for timing loops)."""
    import jax
    from jax.sharding import Mesh, PartitionSpec
    from jax.experimental.shard_map import shard_map
    from concourse import bass2jax

    global _nc_cache
    if _nc_cache is None:
        _nc_cache = _build()
    nc = _nc_cache
    in_maps = _host_inputs(**inputs)

    bass2jax.install_neuronx_cc_hook()
    partition_name = nc.partition_id_tensor.name if nc.partition_id_tensor else None
    in_names, out_names, out_avals, zero_outs = [], [], [], []
    for alloc in nc.m.functions[0].allocations:
        if not isinstance(alloc, mybir.MemoryLocationSet):
            continue
        name = alloc.memorylocations[0].name
        if alloc.kind == "ExternalInput":
            if name != partition_name:
                in_names.append(name)
        elif alloc.kind == "ExternalOutput":
            out_names.append(name)
            shape = tuple(alloc.tensor_shape)
            dtype = mybir.dt.np(alloc.dtype)
            out_avals.append(jax.core.ShapedArray(shape, dtype))
            zero_outs.append(np.zeros(shape, dtype))
    n_params = len(in_names)
    all_in = in_names + out_names
    if partition_name is not None:
        all_in.append(partition_name)

    def _body(*args):
        operands = list(args)
        if partition_name is not None:
            operands.append(bass2jax.partition_id_tensor())
        outs = bass2jax._bass_exec_p.bind(
            *operands,
            out_avals=tuple(out_avals),
            in_names=tuple(all_in[: n_params + len(out_names) + (0 if partition_name is None else 1)]),
            out_names=tuple(out_names),
            lowering_input_output_aliases=(),
            sim_require_finite=True,
            sim_require_nnan=True,
            nc=nc,
        )
        return tuple(outs)

    devices = jax.devices()[:NCORES]
    mesh = Mesh(np.asarray(devices), ("core",))
    nin = n_params + len(out_names)
    sharded = jax.jit(
        shard_map(
            _body, mesh=mesh,
            in_specs=(PartitionSpec("core"),) * nin,
            out_specs=(PartitionSpec("core"),) * len(out_names),
            check_rep=False,
        ),
        keep_unused=True,
    )
    concat_in = [
        np.concatenate([np.asarray(in_maps[c][nm]) for c in range(NCORES)], axis=0)
        for nm in in_names
    ]
    concat_zeros = [
        np.zeros((NCORES * z.shape[0], *z.shape[1:]), z.dtype) for z in zero_outs
    ]
    from jax.sharding import NamedSharding
    sh = NamedSharding(mesh, PartitionSpec("core"))
    args = [jax.device_put(a, sh) for a in concat_in + concat_zeros]

    def run():
        outs = sharded(*args)
        jax.block_until_ready(outs)
        return outs

    def run_async():
        return sharded(*args)

    def assemble(outs):
        res = np.asarray(outs[out_names.index("outT")]).reshape(NCORES, D, T)
        out = np.empty((B, S, D), np.float32)
        for core in range(NCORES):
            b, j = core // CH, core % CH
            for i in range(NT):
                g = j + CH * i
                out[b, P * g : P * (g + 1), :] = res[core][:, P * i : P * (i + 1)].T
        return out

    run.run_async = run_async
    return run, assemble
